# revision 3
# baseline (speedup 1.0000x reference)
"""Explorer GNN message-passing kernel for 8 TRN2 NeuronCores (Bass/Tile).

Strategy (node-sharded, edge-local):
  - Nodes split contiguously across 8 cores (NODE_LOC each). Each core owns
    every edge whose dst falls in its range, so segment-max is core-local.
  - Per core, owned nodes are permuted by ascending in-degree into "slots"
    (blocks of 128). Edges are laid out in (round, block, partition) order so
    that one round-tile of 128..512 messages max-combines into a contiguous
    column range of a feature-major SBUF accumulator with a single DVE
    tensor_tensor(max) - no scatter hardware needed. Pad slots duplicate a
    real edge of the same node (max is idempotent -> exact); zero-degree
    nodes get a -1e30 additive mask on the few affected tiles.
  - Per iteration the cores exchange exactly the x-rows each side needs via
    per-pair request lists + one AllToAll; the receive buffer is small enough
    (< 32768 rows) to index with int16, enabling the fast dma_gather path.
  - The concat-MLPs are refactored into per-operand folded weights:
      mlp2([xj-xi, xj, xi, y]) = xj@(W0+W1) + xi@(W2-W0) + y@W3 + ...
    fx's second bias is hoisted out of the segment-max (max(m_i)+b2), and the
    edge state y is stored shifted by fy_b2 so fy's running max needs no bias.
"""

import os
import sys
import numpy as np

import concourse.bass as bass
import concourse.mybir as mybir
import concourse.bacc as bacc
import concourse.tile as tile
from concourse.bass_utils import run_bass_kernel_spmd
from concourse.masks import make_identity

NCORE = 8
P = 128
H = 64
TILE_W = 512
GC = 1024  # max indices per dma_gather call (HW SWDGE ring limit is < 2048)
NEG = -1.0e30
F32 = mybir.dt.float32
I16 = mybir.dt.int16

LAST_EXEC_NS = None
LAST_TRACE = None
_BUILD_CACHE = {}
SIM_SINGLE = False  # build single-core variant (collective -> DMA) for TimelineSim
MM_F32R = os.environ.get("MM_F32R", "0") == "1"  # relaxed-precision matmuls (1.5 vs 2.0 cyc/row)
F32R = mybir.dt.float32r


def _log(msg):
    print(f"[kernel] {msg}", file=sys.stderr, flush=True)


def _wrap16(ids, ncols):
    """Wrap an index list into the [16, ncols] dma_gather layout."""
    out = np.zeros((16, ncols), dtype=np.int16)
    n = len(ids)
    out[np.arange(n) % 16, np.arange(n) // 16] = ids.astype(np.int16)
    return out


def _wrap16_chunks(ids, chunk):
    """Wrap an index list chunk-by-chunk (one dma_gather call per chunk)."""
    n = len(ids)
    out = np.zeros((16, n // 16), dtype=np.int16)
    off = 0
    while off < n:
        m = min(chunk, n - off)
        out[:, off // 16:(off + m) // 16] = _wrap16(ids[off:off + m], m // 16)
        off += m
    return out


def _preprocess(v, labels, edge_index):
    N, C = v.shape
    D = C + 2
    E = edge_index.shape[1]
    NODE_LOC = (N + NCORE - 1) // NCORE
    NBLK = (NODE_LOC + P - 1) // P
    S_NODE = NBLK * P

    vc = np.concatenate([v, labels], axis=1).astype(np.float32)  # [N, D]
    gi = int(np.argmax(labels[:, 1]))
    goal = vc[gi]
    d = vc - goal
    feat36 = np.concatenate(
        [vc, np.broadcast_to(goal, vc.shape), d, d * d], axis=1
    ).astype(np.float32)  # [N, 4D]

    src = edge_index[0].astype(np.int64)
    dst = edge_index[1].astype(np.int64)
    owner = dst // NODE_LOC

    cores = []
    for c in range(NCORE):
        lo, hi = c * NODE_LOC, min((c + 1) * NODE_LOC, N)
        nloc = hi - lo
        eids = np.where(owner == c)[0]
        dl = dst[eids] - lo
        deg = np.bincount(dl, minlength=nloc)
        order = np.argsort(deg, kind="stable")  # ascending degree
        slot_of_local = np.empty(nloc, dtype=np.int64)
        slot_of_local[order] = np.arange(nloc)
        # CSR of edges by local dst
        es = eids[np.argsort(dl, kind="stable")]
        rp = np.zeros(nloc + 1, dtype=np.int64)
        rp[1:] = np.cumsum(deg)
        # per-slot padded arrays
        deg_s = np.zeros(S_NODE, dtype=np.int64)
        deg_s[:nloc] = deg[order]
        node_s = np.full(S_NODE, -1, dtype=np.int64)
        node_s[:nloc] = order + lo  # global node id per slot
        rp_s = np.zeros(S_NODE, dtype=np.int64)
        rp_s[:nloc] = rp[order]
        Rb = np.zeros(NBLK, dtype=np.int64)
        for b in range(NBLK):
            Rb[b] = deg_s[b * P:(b + 1) * P].max()
        cores.append(
            dict(lo=lo, nloc=nloc, deg_s=deg_s, node_s=node_s, rp_s=rp_s,
                 es=es, Rb=Rb, slot_of_local=slot_of_local)
        )

    Rb = np.max(np.stack([cc["Rb"] for cc in cores]), axis=0)  # [NBLK]
    assert np.all(np.diff(Rb) >= 0), "Rb must be nondecreasing (ascending degree sort)"
    maxR = int(Rb.max())

    # tile structure (uniform across cores): per round, chunk the block-suffix
    tiles = []  # (r, col0, w, sbase)
    sbase = 0
    for r in range(maxR):
        b_r = int(np.searchsorted(Rb, r + 1))  # first block with Rb > r
        col0 = b_r * P
        wtot = (NBLK - b_r) * P
        off = 0
        while off < wtot:
            w = min(TILE_W, wtot - off)
            tiles.append((r, col0 + off, w, sbase + off))
            off += w
        sbase += wtot
    S_E = sbase
    n_sub = S_E // P

    # per-slot edge assignment (per core)
    slot_edge = np.full((NCORE, S_E), -1, dtype=np.int64)  # edge id or -1
    slot_col = np.empty(S_E, dtype=np.int64)  # acc column of each slot
    spos = 0
    for r in range(maxR):
        b_r = int(np.searchsorted(Rb, r + 1))
        cols = np.arange(b_r * P, NBLK * P)
        n_s = len(cols)
        slot_col[spos:spos + n_s] = cols
        for c in range(NCORE):
            cc = cores[c]
            degc = cc["deg_s"][cols]
            rpc = cc["rp_s"][cols]
            has = degc > r
            dup = (~has) & (degc > 0)
            e = np.full(n_s, -1, dtype=np.int64)
            e[has] = cc["es"][rpc[has] + r]
            e[dup] = cc["es"][rpc[dup]]
            slot_edge[c, spos:spos + n_s] = e
        spos += n_s
    assert spos == S_E

    # masked subtiles: any core has a pad slot (-1 edge) on a REAL node there
    sub_masked = np.zeros(n_sub, dtype=bool)
    for si in range(n_sub):
        cols = slot_col[si * P: si * P + P]
        for c in range(NCORE):
            cc = cores[c]
            e = slot_edge[c, si * P: si * P + P]
            real = cc["node_s"][cols] >= 0
            if np.any((e < 0) & real):
                sub_masked[si] = True
                break
    masked_ids = np.where(sub_masked)[0]
    mask_index = {int(s): i for i, s in enumerate(masked_ids)}
    NMASK = max(1, len(masked_ids))

    # request lists and receive-position maps
    # req[c][d] = sorted unique src nodes of core c's edges owned by core d
    req = [[None] * NCORE for _ in range(NCORE)]
    maxlen = 0
    for c in range(NCORE):
        e = slot_edge[c]
        srcs = np.unique(src[e[e >= 0]])
        bounds = np.searchsorted(srcs, np.arange(1, NCORE) * NODE_LOC)
        parts = np.split(srcs, bounds)
        for dd in range(NCORE):
            req[c][dd] = parts[dd]
            maxlen = max(maxlen, len(parts[dd]))
    R = ((maxlen + P - 1) // P) * P
    RJ = R // P

    per_core_inputs = []
    meta = dict(N=N, C=C, D=D, E=E, NODE_LOC=NODE_LOC, NBLK=NBLK,
                S_NODE=S_NODE, S_E=S_E, maxR=maxR, tiles=tiles,
                masked_ids=masked_ids.tolist(), mask_index=mask_index,
                NMASK=NMASK, R=R, n_sub=n_sub)

    for c in range(NCORE):
        cc = cores[c]
        # receive-position map: node -> recvbuf row
        posmap = np.zeros(N, dtype=np.int64)
        for dd in range(NCORE):
            lst = req[c][dd]
            i = np.arange(len(lst))
            kc = i // GC
            i2 = i % GC
            posmap[lst] = dd * R + (i2 % P) * RJ + kc * (GC // P) + i2 // P
        e = slot_edge[c]
        has_e = e >= 0
        srcn = np.where(has_e, src[np.clip(e, 0, None)], 0)
        srcpos = np.where(has_e, posmap[srcn], 0)
        # slot gather indices, wrapped per GC-call
        ncols16 = S_E // 16
        slotidx = np.zeros((16, ncols16), dtype=np.int16)
        base = 0
        while base < S_E:
            n = min(GC, S_E - base)
            w = _wrap16(srcpos[base:base + n], n // 16)
            slotidx[:, base // 16: (base + n) // 16] = w
            base += n
        slotidx_full = np.tile(slotidx, (8, 1))  # [128, S_E/16]

        # send gather indices: my local myslice rows for each dest's request of me
        sendidx = np.zeros((16, NCORE * R // 16), dtype=np.int16)
        for dd in range(NCORE):  # dd = destination core requesting from me
            lst = req[dd][c]
            rows = cc["slot_of_local"][lst - cc["lo"]]
            rows = np.concatenate([rows, np.zeros(R - len(rows), dtype=np.int64)])
            sendidx[:, dd * (R // 16): (dd + 1) * (R // 16)] = _wrap16_chunks(rows, GC)
        sendidx_full = np.tile(sendidx, (8, 1))

        # rhs18 for hy: rows 0:9 = vc[src(e)], rows 9:18 = vc[dst(e)]
        rhs18 = np.zeros((2 * D, S_E), dtype=np.float32)
        dstn = np.where(has_e, dst[np.clip(e, 0, None)], 0)
        rhs18[:D, has_e] = vc[srcn[has_e]].T
        rhs18[D:, has_e] = vc[dstn[has_e]].T

        # mask data [64, NMASK*128]
        mask64 = np.zeros((H, NMASK * P), dtype=np.float32)
        for i, si in enumerate(masked_ids):
            cols = slot_col[si * P: si * P + P]
            ee = slot_edge[c, si * P: si * P + P]
            real = cc["node_s"][cols] >= 0
            dead = (ee < 0) & real
            mask64[:, i * P: (i + 1) * P][:, dead] = NEG

        # node-init features, slot order, transposed
        f36 = np.zeros((feat36.shape[1], S_NODE), dtype=np.float32)
        realn = cc["node_s"] >= 0
        f36[:, realn] = feat36[cc["node_s"][realn]].T

        per_core_inputs.append(dict(
            slotidx=slotidx_full, sendidx=sendidx_full, rhs18=rhs18,
            mask64=mask64, feat36T=f36,
        ))

    meta["slot_col"] = slot_col
    meta["cores"] = cores
    return meta, per_core_inputs


def _fold_weights(w):
    """Host-side weight refactoring. w = dict of reference weights (np.float32)."""
    out = {}
    f = lambda a: np.ascontiguousarray(a, dtype=np.float32)
    out["hx_w1"] = f(w["hx_w1"])                      # [4D, 64]
    out["hx_w2"] = f(w["hx_w2"])
    out["hx_b1"] = f(w["hx_b1"][:, None])
    out["hx_b2"] = f(w["hx_b2"][:, None])
    D = w["hy_w1"].shape[0] // 3
    U = w["hy_w1"]
    out["hyAB"] = f(np.vstack([U[2 * D:3 * D] - U[0:D],      # vi = vc[src]
                               U[0:D] + U[D:2 * D]]))        # vj = vc[dst]
    out["hy_w2"] = f(w["hy_w2"])
    out["hy_b1"] = f(w["hy_b1"][:, None])
    out["hy_b2eff"] = f((w["hy_b2"] - w["fy_b2"])[:, None])  # y stored shifted by fy_b2
    W = w["fx_w1"]
    fxA = W[64:128] + W[0:64]               # xj = x[src] (gathered)
    fxB = W[128:192] - W[0:64]              # xi = x[dst] (own)
    out["fxAB"] = f(np.vstack([fxA, fxB]))  # K=128 stacked vs rhs [xjT; xown]
    out["fxC"] = f(W[192:256])              # y~
    out["fx_w2"] = f(w["fx_w2"])
    out["fx_b1eff"] = f((w["fx_b1"] + w["fy_b2"] @ W[192:256])[:, None])
    out["fx_b2"] = f(w["fx_b2"][:, None])
    V = w["fy_w1"]
    fyB = V[128:192] - V[0:64]              # xi = x[src] (gathered)
    fyA = V[0:64] + V[64:128]               # xj = x[dst] (own)
    out["fyBA"] = f(np.vstack([fyB, fyA]))
    out["fy_w2"] = f(w["fy_w2"])
    out["fy_b1"] = f(w["fy_b1"][:, None])
    out["feta_w1"] = f(w["feta_w1"])
    out["feta_w2"] = f(w["feta_w2"])
    out["feta_w3"] = f(w["feta_w3"])
    out["feta_b1"] = f(w["feta_b1"][:, None])
    out["feta_b2"] = f(w["feta_b2"][:, None])
    return out


def _build(meta, wshapes, LOOP):
    S_NODE, S_E, NBLK = meta["S_NODE"], meta["S_E"], meta["NBLK"]
    NMASK, R = meta["NMASK"], meta["R"]
    RJ = R // P
    tiles = meta["tiles"]
    mask_index = meta["mask_index"]
    n_calls = (S_E + GC - 1) // GC

    nc = bacc.Bacc("TRN2", target_bir_lowering=False, debug=False,
                   num_devices=1 if SIM_SINGLE else NCORE,
                   num_swdge_queues=4)

    # ---- inputs ----
    din = {}
    for name, shp in wshapes.items():
        din[name] = nc.dram_tensor(name, list(shp), F32, kind="ExternalInput")
    feat36T = nc.dram_tensor("feat36T", [wshapes["hx_w1"][0], S_NODE], F32, kind="ExternalInput")
    rhs18 = nc.dram_tensor("rhs18", [wshapes["hyAB"][0], S_E], F32, kind="ExternalInput")
    slotidx = nc.dram_tensor("slotidx", [P, S_E // 16], I16, kind="ExternalInput")
    sendidx = nc.dram_tensor("sendidx", [P, NCORE * R // 16], I16, kind="ExternalInput")
    mask64 = nc.dram_tensor("mask64", [H, NMASK * P], F32, kind="ExternalInput")

    outslots = nc.dram_tensor("outslots", [S_NODE, 1], F32, kind="ExternalOutput")

    # ---- internal DRAM ----
    yT = nc.dram_tensor("yT", [H, S_E], F32)
    myslice = nc.dram_tensor("myslice", [S_NODE, H], F32)
    sendbuf = nc.dram_tensor("sendbuf", [NCORE * R, H], F32)
    recvbuf = nc.dram_tensor("recvbuf", [NCORE * R, H], F32)

    myslice_pview = myslice.ap().rearrange("(b p) f -> p b f", p=P)
    outslots_pview = outslots.ap().rearrange("(b p) o -> p b o", p=P)

    ACT = mybir.ActivationFunctionType
    ALU = mybir.AluOpType

    with tile.TileContext(nc) as tc:
        with (
            tc.tile_pool(name="persist", bufs=1) as pp,
            tc.tile_pool(name="callbuf", bufs=8) as cbp,
            tc.tile_pool(name="work", bufs=3) as wp,
            tc.tile_pool(name="sendp", bufs=2) as sp,
            tc.tile_pool(name="pz", bufs=6, space="PSUM") as pz,
            tc.tile_pool(name="ptr", bufs=2, space="PSUM") as ptr,
        ):
            # ---- persistent tiles ----
            ident = pp.tile([P, P], F32, tag="ident")
            make_identity(nc, ident[:])
            W = {}
            for name, shp in wshapes.items():
                t = pp.tile(list(shp), F32, tag=f"w_{name}")
                nc.sync.dma_start(out=t[:], in_=din[name][:, :])
                W[name] = t
            xown = pp.tile([H, S_NODE], F32, tag="xown")
            acc = pp.tile([H, S_NODE], F32, tag="acc")
            staging = pp.tile([P, NBLK * H], F32, tag="staging")
            staging2 = pp.tile([P, NBLK], F32, tag="staging2")
            sidx = pp.tile([P, S_E // 16], I16, tag="sidx")
            nc.sync.dma_start(out=sidx[:], in_=slotidx[:, :])
            kidx = pp.tile([P, NCORE * R // 16], I16, tag="kidx")
            nc.sync.dma_start(out=kidx[:], in_=sendidx[:, :])
            msk = pp.tile([H, NMASK * P], F32, tag="msk")
            nc.sync.dma_start(out=msk[:], in_=mask64[:, :])

            def mm(out, lhsT, rhs, start, stop):
                nc.tensor.matmul(out[:] if hasattr(out, "tile") else out,
                                 lhsT, rhs, start=start, stop=stop)

            def MM(out_ap, lhsT_ap, rhs_ap, start, stop):
                if MM_F32R:
                    lhsT_ap = lhsT_ap.bitcast(F32R)
                    rhs_ap = rhs_ap.bitcast(F32R)
                nc.tensor.matmul(out_ap, lhsT_ap, rhs_ap, start=start, stop=stop)

            evac_ct = [0]

            def evac(dst_ap, src_ap):
                # alternate ACT / DVE to balance engines
                if evac_ct[0] % 2 == 0:
                    nc.scalar.copy(out=dst_ap, in_=src_ap)
                else:
                    nc.vector.tensor_copy(out=dst_ap, in_=src_ap)
                evac_ct[0] += 1

            # ---------- readback: xown -> staging -> myslice ----------
            def readback():
                for b in range(NBLK):
                    ps = ptr.tile([P, H], F32, tag="ptr")
                    nc.tensor.transpose(
                        out=ps[:], in_=xown[:, b * P:(b + 1) * P],
                        identity=ident[0:H, 0:H])
                    evac(staging[:, b * H:(b + 1) * H], ps[:])
                nc.sync.dma_start(
                    out=myslice_pview,
                    in_=staging[:].rearrange("p (b f) -> p b f", b=NBLK))

            # ---------- exchange: myslice -> sendbuf -> A2A -> recvbuf ----------
            def exchange():
                qq = [0]
                for dd in range(NCORE):
                    st = sp.tile([P, RJ, H], F32, tag="sendt")
                    off = 0
                    while off < R:
                        n = min(GC, R - off)
                        nc.gpsimd.dma_gather(
                            out_ap=st[:, off // P:(off + n) // P, :],
                            in_ap=myslice[:, :],
                            idxs_ap=kidx[:, (dd * R + off) // 16:(dd * R + off + n) // 16],
                            num_idxs=n, num_idxs_reg=n, elem_size=H,
                            queue_num=qq[0] % 4)
                        qq[0] += 1
                        off += n
                    dv = sendbuf.ap()[dd * R:(dd + 1) * R, :].rearrange(
                        "(p j) f -> p (j f)", p=P)
                    nc.sync.dma_start(out=dv, in_=st[:].rearrange("p j f -> p (j f)"))
                if SIM_SINGLE:
                    nc.sync.dma_start(out=recvbuf.ap().rearrange(
                        "(p a) f -> p (a f)", p=P),
                        in_=sendbuf.ap().rearrange("(p a) f -> p (a f)", p=P))
                else:
                    nc.gpsimd.collective_compute(
                        "AllToAll", ALU.bypass,
                        replica_groups=[list(range(NCORE))],
                        ins=[sendbuf.ap()], outs=[recvbuf.ap()])

            # ---------- slot gather: recvbuf -> callbufs ----------
            def slot_gather():
                bufs = []
                base = 0
                qn = 0
                while base < S_E:
                    n = min(GC, S_E - base)
                    st = cbp.tile([P, GC // P, H], F32, tag="cb")
                    nc.gpsimd.dma_gather(
                        out_ap=st[:, : n // P, :], in_ap=recvbuf[:, :],
                        idxs_ap=sidx[:, base // 16:(base + n) // 16],
                        num_idxs=n, num_idxs_reg=n, elem_size=H,
                        queue_num=qn % 4)
                    qn += 1
                    bufs.append(st)
                    base += n
                return bufs

            # ---------- stacked rhs tile: rows 0:64 = xjT, rows 64:128 = xown ----------
            def make_stk(bufs, sbase_, col0, w):
                stk = wp.tile([P, TILE_W], F32, tag="stk")
                for j in range(w // P):
                    s = sbase_ + j * P
                    ci, wc = (s // GC), (s % GC) // P
                    g = bufs[ci][:, wc, :]
                    ps = ptr.tile([H, P], F32, tag="ptr")
                    nc.tensor.transpose(out=ps[:], in_=g, identity=ident[:])
                    evac(stk[0:H, j * P:(j + 1) * P], ps[:])
                nc.sync.dma_start(out=stk[H:P, :w], in_=xown[:, col0:col0 + w])
                return stk

            # ---------- fused phase: fy(k-1) then fx(k), sharing xjT ----------
            def fused_phase(bufs, with_fy, write_y):
                nc.vector.tensor_tensor(
                    out=acc[:], in0=xown[:],
                    in1=W["fx_b2"][:, :1].to_broadcast([H, S_NODE]),
                    op=ALU.subtract)
                for (r, col0, w, sbase_) in tiles:
                    stk = make_stk(bufs, sbase_, col0, w)
                    yt = wp.tile([H, TILE_W], F32, tag="yt")
                    nc.sync.dma_start(out=yt[:, :w], in_=yT[:, sbase_:sbase_ + w])
                    if with_fy:
                        z1y = pz.tile([H, TILE_W], F32, tag="z")
                        MM(z1y[:, :w], W["fyBA"][:], stk[:, :w], True, True)
                        h1y = wp.tile([H, TILE_W], F32, tag="h1")
                        nc.scalar.activation(out=h1y[:, :w], in_=z1y[:, :w],
                                             func=ACT.Relu, bias=W["fy_b1"][:, :1])
                        z2y = pz.tile([H, TILE_W], F32, tag="z")
                        MM(z2y[:, :w], W["fy_w2"][:], h1y[:, :w], True, True)
                        nc.vector.tensor_tensor(out=yt[:, :w], in0=yt[:, :w],
                                                in1=z2y[:, :w], op=ALU.max)
                        if write_y:
                            nc.sync.dma_start(out=yT[:, sbase_:sbase_ + w],
                                              in_=yt[:, :w])
                    z1 = pz.tile([H, TILE_W], F32, tag="z")
                    MM(z1[:, :w], W["fxAB"][:], stk[:, :w], True, False)
                    MM(z1[:, :w], W["fxC"][:], yt[:, :w], False, True)
                    h1 = wp.tile([H, TILE_W], F32, tag="h1")
                    nc.scalar.activation(out=h1[:, :w], in_=z1[:, :w],
                                         func=ACT.Relu, bias=W["fx_b1eff"][:, :1])
                    z2 = pz.tile([H, TILE_W], F32, tag="z")
                    MM(z2[:, :w], W["fx_w2"][:], h1[:, :w], True, True)
                    # max into acc, applying mask on flagged subtiles
                    j = 0
                    while j < w // P:
                        gsub = (sbase_ + j * P) // P
                        if gsub in mask_index:
                            mi = mask_index[gsub]
                            tmp = wp.tile([H, P], F32, tag="mtmp")
                            nc.vector.tensor_tensor(
                                out=tmp[:], in0=z2[:, j * P:(j + 1) * P],
                                in1=msk[:, mi * P:(mi + 1) * P], op=ALU.add)
                            nc.vector.tensor_tensor(
                                out=acc[:, col0 + j * P:col0 + (j + 1) * P],
                                in0=acc[:, col0 + j * P:col0 + (j + 1) * P],
                                in1=tmp[:], op=ALU.max)
                            j += 1
                        else:
                            j2 = j
                            while j2 < w // P and ((sbase_ + j2 * P) // P) not in mask_index:
                                j2 += 1
                            nc.vector.tensor_tensor(
                                out=acc[:, col0 + j * P:col0 + j2 * P],
                                in0=acc[:, col0 + j * P:col0 + j2 * P],
                                in1=z2[:, j * P:j2 * P], op=ALU.max)
                            j = j2
                # combine: xown = acc + fx_b2
                nc.vector.tensor_tensor(
                    out=xown[:], in0=acc[:],
                    in1=W["fx_b2"][:, :1].to_broadcast([H, S_NODE]),
                    op=ALU.add)

            # ---------- init: hx ----------
            K36 = wshapes["hx_w1"][0]
            off = 0
            while off < S_NODE:
                w = min(TILE_W, S_NODE - off)
                ft = wp.tile([K36, TILE_W], F32, tag="ft")
                nc.sync.dma_start(out=ft[:, :w], in_=feat36T[:, off:off + w])
                z1 = pz.tile([H, TILE_W], F32, tag="z")
                MM(z1[:, :w], W["hx_w1"][:], ft[:, :w], True, True)
                h1 = wp.tile([H, TILE_W], F32, tag="h1")
                nc.scalar.activation(out=h1[:, :w], in_=z1[:, :w],
                                     func=ACT.Relu, bias=W["hx_b1"][:, :1])
                z2 = pz.tile([H, TILE_W], F32, tag="z")
                MM(z2[:, :w], W["hx_w2"][:], h1[:, :w], True, True)
                nc.scalar.activation(out=xown[:, off:off + w], in_=z2[:, :w],
                                     func=ACT.Identity, bias=W["hx_b2"][:, :1])
                off += w

            # ---------- init: hy ----------
            K18 = wshapes["hyAB"][0]
            for (r, col0, w, sbase_) in tiles:
                r18 = wp.tile([K18, TILE_W], F32, tag="r18")
                nc.sync.dma_start(out=r18[:, :w], in_=rhs18[:, sbase_:sbase_ + w])
                z1 = pz.tile([H, TILE_W], F32, tag="z")
                MM(z1[:, :w], W["hyAB"][:], r18[:, :w], True, True)
                h1 = wp.tile([H, TILE_W], F32, tag="h1")
                nc.scalar.activation(out=h1[:, :w], in_=z1[:, :w],
                                     func=ACT.Relu, bias=W["hy_b1"][:, :1])
                z2 = pz.tile([H, TILE_W], F32, tag="z")
                MM(z2[:, :w], W["hy_w2"][:], h1[:, :w], True, True)
                yt = wp.tile([H, TILE_W], F32, tag="yt")
                nc.scalar.activation(out=yt[:, :w], in_=z2[:, :w],
                                     func=ACT.Identity, bias=W["hy_b2eff"][:, :1])
                nc.sync.dma_start(out=yT[:, sbase_:sbase_ + w], in_=yt[:, :w])

            # ---------- initial exchange of x0 ----------
            readback()
            exchange()

            # ---------- iterations ----------
            for k in range(LOOP):
                bufs = slot_gather()
                fused_phase(bufs, with_fy=(k > 0), write_y=(k < LOOP - 1))
                if k < LOOP - 1:
                    readback()
                    exchange()

            # ---------- final MLP ----------
            off = 0
            while off < S_NODE:
                w = min(TILE_W, S_NODE - off)
                z1 = pz.tile([H, TILE_W], F32, tag="z")
                MM(z1[:, :w], W["feta_w1"][:], xown[:, off:off + w], True, True)
                h1 = wp.tile([H, TILE_W], F32, tag="h1")
                nc.scalar.activation(out=h1[:, :w], in_=z1[:, :w],
                                     func=ACT.Relu, bias=W["feta_b1"][:, :1])
                z2 = pz.tile([H, TILE_W], F32, tag="z")
                MM(z2[:, :w], W["feta_w2"][:], h1[:, :w], True, True)
                h2 = wp.tile([H, TILE_W], F32, tag="h2")
                nc.scalar.activation(out=h2[:, :w], in_=z2[:, :w],
                                     func=ACT.Relu, bias=W["feta_b2"][:, :1])
                for j in range(w // P):
                    b = (off + j * P) // P
                    ps = ptr.tile([P, H], F32, tag="ptr")
                    nc.tensor.matmul(ps[:, 0:1], h2[:, j * P:(j + 1) * P],
                                     W["feta_w3"][:], start=True, stop=True)
                    evac(staging2[:, b:b + 1], ps[:, 0:1])
                off += w
            nc.sync.dma_start(
                out=outslots_pview,
                in_=staging2[:].rearrange("p (b o) -> p b o", b=NBLK))

    _log(f"built program: {S_E=} {len(tiles)=} masks={NMASK} R={R}")
    nc.compile()
    _log("compiled")
    return nc


def kernel(**inputs):
    global LAST_EXEC_NS
    v = np.asarray(inputs["v"], dtype=np.float32)
    labels = np.asarray(inputs["labels"], dtype=np.float32)
    edge_index = np.asarray(inputs["edge_index"]).astype(np.int64)
    LOOP = int(np.asarray(inputs["loop"]))

    import hashlib
    ck = hashlib.sha1(edge_index.tobytes()).hexdigest() + f"_{LOOP}_{v.shape}"
    if ck in _BUILD_CACHE:
        meta, pci, nc = _BUILD_CACHE[ck]
    else:
        meta, pci, nc = None, None, None
    if meta is None:
        meta, pci = _preprocess(v, labels, edge_index)
    wnames = ["hx_w1", "hx_w2", "hx_b1", "hx_b2", "hyAB", "hy_w2", "hy_b1",
              "hy_b2eff", "fxAB", "fxC", "fx_w2", "fx_b1eff", "fx_b2",
              "fyBA", "fy_w2", "fy_b1",
              "feta_w1", "feta_w2", "feta_w3", "feta_b1", "feta_b2"]
    wf = _fold_weights({k: np.asarray(val, dtype=np.float32)
                        for k, val in inputs.items()
                        if k not in ("v", "labels", "edge_index", "loop")})
    wshapes = {n: wf[n].shape for n in wnames}

    if nc is None:
        nc = _build(meta, wshapes, LOOP)
        _BUILD_CACHE[ck] = (meta, pci, nc)

    in_maps = []
    for c in range(NCORE):
        m = {n: wf[n] for n in wnames}
        m["feat36T"] = pci[c]["feat36T"]
        m["rhs18"] = pci[c]["rhs18"]
        m["slotidx"] = pci[c]["slotidx"]
        m["sendidx"] = pci[c]["sendidx"]
        m["mask64"] = pci[c]["mask64"]
        in_maps.append(m)

    res = run_bass_kernel_spmd(nc, in_maps, core_ids=list(range(NCORE)),
                               tmpdir=os.environ.get("BASS_TMPDIR") or None)
    LAST_EXEC_NS = res.exec_time_ns
    global LAST_TRACE
    LAST_TRACE = res.instructions_and_trace

    N = meta["N"]
    NODE_LOC = meta["NODE_LOC"]
    out = np.zeros((N, 1), dtype=np.float32)
    for c in range(NCORE):
        cc = meta["cores"][c]
        slots = cc["slot_of_local"]  # [nloc]
        vals = res.results[c]["outslots"][:, 0]
        out[cc["lo"]:cc["lo"] + cc["nloc"], 0] = vals[slots]
    return out



# revision 36
# speedup vs baseline: 1.6817x; 1.6817x over previous
"""Explorer GNN message-passing kernel for 8 TRN2 NeuronCores (Bass/Tile).

Strategy (node-sharded, edge-local), v2:
  - Nodes split contiguously across 8 cores. Each core owns every edge whose
    dst falls in its range, so segment-max is core-local. Owned nodes are
    permuted by ascending in-degree into "slots" (blocks of 128); edges laid
    out in (round, block, partition) order so one round-tile of messages
    max-combines into a contiguous column range of the feature-major node
    state with a single DVE max.
  - Node state is kept as x~ = x - fx_b2 on SBUF partitions 64:128 for the
    whole kernel; all biases are folded so scatter-max and the y-update need
    no bias ops at all.
  - x rows are exchanged between cores as bf16 256-byte rows; the per-edge
    x[src] gather uses dma_gather(transpose=True), which lands the rows
    FEATURE-MAJOR in SBUF - no PE transposes in the inner loop, and all
    inner-loop matmuls run in bf16 (2x PE rate):
      z1  = [fyB|fxA]^T xj~  (+)  [fyA|fxB]^T xi~   (two K=64->M=128 passes)
      z1x += fxC^T y~                                (K=64 quadrant pass)
      z2  = diag(fy_w2, fx_w2)^T [h1y(t); h1x(t-1)]  (one K=128->M=128 pass,
            software-pipelined across tiles)
  - The edge-init MLP (hy) is similarly pipelined into ONE matmul per tile
    with lhsT [[0,hyAB],[hy_w2,0]].
"""

import os
import sys
import numpy as np
import ml_dtypes

import concourse.bass as bass
import concourse.mybir as mybir
import concourse.bacc as bacc
import concourse.tile as tile
from concourse.bass_utils import run_bass_kernel_spmd
from concourse.masks import make_identity

NCORE = 8
P = 128
H = 64
TILE_W = 512
GC = 1024   # max indices per dma_gather call (ring limit < 2048)
# NOTE: dma_gather(transpose=True) silently corrupts data beyond ~16
# back-to-back calls (HW-verified); gathered rows are transposed on the
# PE instead (bf16: 1 cyc/row).
NEG = -1.0e30
F32 = mybir.dt.float32
BF16 = mybir.dt.bfloat16
I16 = mybir.dt.int16
NPBF16 = ml_dtypes.bfloat16

LAST_EXEC_NS = None
LAST_TRACE = None
_BUILD_CACHE = {}
SIM_SINGLE = False  # build single-core variant (collective -> DMA)


def _log(msg):
    print(f"[kernel] {msg}", file=sys.stderr, flush=True)


def _wrap16(ids, ncols):
    out = np.zeros((16, ncols), dtype=np.int16)
    n = len(ids)
    out[np.arange(n) % 16, np.arange(n) // 16] = ids.astype(np.int16)
    return out


def _wrap16_chunks(ids, chunk):
    n = len(ids)
    out = np.zeros((16, n // 16), dtype=np.int16)
    off = 0
    while off < n:
        m = min(chunk, n - off)
        out[:, off // 16:(off + m) // 16] = _wrap16(ids[off:off + m], m // 16)
        off += m
    return out


def _preprocess(v, labels, edge_index):
    N, C = v.shape
    D = C + 2
    E = edge_index.shape[1]
    NODE_LOC = (N + NCORE - 1) // NCORE
    NBLK = (NODE_LOC + P - 1) // P
    S_NODE = NBLK * P

    vc = np.concatenate([v, labels], axis=1).astype(np.float32)  # [N, D]
    gi = int(np.argmax(labels[:, 1]))
    goal = vc[gi]
    d = vc - goal
    feat36 = np.concatenate(
        [vc, np.broadcast_to(goal, vc.shape), d, d * d], axis=1
    ).astype(np.float32)  # [N, 4D]

    src = edge_index[0].astype(np.int64)
    dst = edge_index[1].astype(np.int64)
    owner = dst // NODE_LOC

    cores = []
    for c in range(NCORE):
        lo, hi = c * NODE_LOC, min((c + 1) * NODE_LOC, N)
        nloc = hi - lo
        eids = np.where(owner == c)[0]
        dl = dst[eids] - lo
        deg = np.bincount(dl, minlength=nloc)
        order = np.argsort(deg, kind="stable")  # ascending degree
        slot_of_local = np.empty(nloc, dtype=np.int64)
        slot_of_local[order] = np.arange(nloc)
        es = eids[np.argsort(dl, kind="stable")]
        rp = np.zeros(nloc + 1, dtype=np.int64)
        rp[1:] = np.cumsum(deg)
        deg_s = np.zeros(S_NODE, dtype=np.int64)
        deg_s[:nloc] = deg[order]
        node_s = np.full(S_NODE, -1, dtype=np.int64)
        node_s[:nloc] = order + lo
        rp_s = np.zeros(S_NODE, dtype=np.int64)
        rp_s[:nloc] = rp[order]
        Rb = np.zeros(NBLK, dtype=np.int64)
        for b in range(NBLK):
            Rb[b] = deg_s[b * P:(b + 1) * P].max()
        cores.append(
            dict(lo=lo, nloc=nloc, deg_s=deg_s, node_s=node_s, rp_s=rp_s,
                 es=es, Rb=Rb, slot_of_local=slot_of_local)
        )

    Rb = np.max(np.stack([cc["Rb"] for cc in cores]), axis=0)  # [NBLK]
    assert np.all(np.diff(Rb) >= 0), "Rb must be nondecreasing"
    maxR = int(Rb.max())

    # tile structure (uniform across cores): per round, chunk the block-suffix
    tiles = []  # (r, col0, w, sbase)
    sbase = 0
    for r in range(maxR):
        b_r = int(np.searchsorted(Rb, r + 1))
        col0 = b_r * P
        wtot = (NBLK - b_r) * P
        off = 0
        while off < wtot:
            w = min(TILE_W, wtot - off)
            tiles.append((r, col0 + off, w, sbase + off))
            off += w
        sbase += wtot
    S_E = sbase
    n_sub = S_E // P

    slot_edge = np.full((NCORE, S_E), -1, dtype=np.int64)
    slot_col = np.empty(S_E, dtype=np.int64)
    spos = 0
    for r in range(maxR):
        b_r = int(np.searchsorted(Rb, r + 1))
        cols = np.arange(b_r * P, NBLK * P)
        n_s = len(cols)
        slot_col[spos:spos + n_s] = cols
        for c in range(NCORE):
            cc = cores[c]
            degc = cc["deg_s"][cols]
            rpc = cc["rp_s"][cols]
            has = degc > r
            dup = (~has) & (degc > 0)
            e = np.full(n_s, -1, dtype=np.int64)
            e[has] = cc["es"][rpc[has] + r]
            e[dup] = cc["es"][rpc[dup]]
            slot_edge[c, spos:spos + n_s] = e
        spos += n_s
    assert spos == S_E

    # masked subtiles: any core has a pad slot (-1 edge) on a REAL node there
    sub_masked = np.zeros(n_sub, dtype=bool)
    for si in range(n_sub):
        cols = slot_col[si * P: si * P + P]
        for c in range(NCORE):
            cc = cores[c]
            e = slot_edge[c, si * P: si * P + P]
            real = cc["node_s"][cols] >= 0
            if np.any((e < 0) & real):
                sub_masked[si] = True
                break
    masked_ids = np.where(sub_masked)[0]
    mask_index = {int(s): i for i, s in enumerate(masked_ids)}
    NMASK = max(1, len(masked_ids))

    # request lists: req[c][d] = sorted unique src nodes of core c's edges owned by d
    req = [[None] * NCORE for _ in range(NCORE)]
    maxlen = 0
    for c in range(NCORE):
        e = slot_edge[c]
        srcs = np.unique(src[e[e >= 0]])
        bounds = np.searchsorted(srcs, np.arange(1, NCORE) * NODE_LOC)
        parts = np.split(srcs, bounds)
        for dd in range(NCORE):
            req[c][dd] = parts[dd]
            maxlen = max(maxlen, len(parts[dd]))
    R = ((maxlen + P - 1) // P) * P
    RJ = R // P

    per_core_inputs = []
    meta = dict(N=N, C=C, D=D, E=E, NODE_LOC=NODE_LOC, NBLK=NBLK,
                S_NODE=S_NODE, S_E=S_E, maxR=maxR, tiles=tiles,
                masked_ids=masked_ids.tolist(), mask_index=mask_index,
                NMASK=NMASK, R=R, n_sub=n_sub)

    for c in range(NCORE):
        cc = cores[c]
        # receive-position map: node -> recvbuf row
        posmap = np.zeros(N, dtype=np.int64)
        for dd in range(NCORE):
            lst = req[c][dd]
            i = np.arange(len(lst))
            kc = i // GC
            i2 = i % GC
            posmap[lst] = dd * R + (i2 % P) * RJ + kc * (GC // P) + i2 // P
        e = slot_edge[c]
        has_e = e >= 0
        srcn = np.where(has_e, src[np.clip(e, 0, None)], 0)
        srcpos = np.where(has_e, posmap[srcn], 0)
        ncols16 = S_E // 16
        slotidx = np.zeros((16, ncols16), dtype=np.int16)
        base = 0
        while base < S_E:
            n = min(GC, S_E - base)
            w = _wrap16(srcpos[base:base + n], n // 16)
            slotidx[:, base // 16: (base + n) // 16] = w
            base += n
        slotidx_full = np.tile(slotidx, (8, 1))  # [128, S_E/16]

        sendidx = np.zeros((16, NCORE * R // 16), dtype=np.int16)
        for dd in range(NCORE):
            lst = req[dd][c]
            rows = cc["slot_of_local"][lst - cc["lo"]]
            rows = np.concatenate([rows, np.zeros(R - len(rows), dtype=np.int64)])
            sendidx[:, dd * (R // 16): (dd + 1) * (R // 16)] = _wrap16_chunks(rows, GC)
        sendidx_full = np.tile(sendidx, (8, 1))

        # rhs18 for hy (bf16): rows 0:9 = vc[src(e)], rows 9:18 = vc[dst(e)]
        rhs18 = np.zeros((2 * D, S_E), dtype=np.float32)
        dstn = np.where(has_e, dst[np.clip(e, 0, None)], 0)
        rhs18[:D, has_e] = vc[srcn[has_e]].T
        rhs18[D:, has_e] = vc[dstn[has_e]].T

        # mask data [64, NMASK*128]
        mask64 = np.zeros((H, NMASK * P), dtype=np.float32)
        for i, si in enumerate(masked_ids):
            cols = slot_col[si * P: si * P + P]
            ee = slot_edge[c, si * P: si * P + P]
            real = cc["node_s"][cols] >= 0
            dead = (ee < 0) & real
            mask64[:, i * P: (i + 1) * P][:, dead] = NEG

        # node-init features, slot order, transposed
        f36 = np.zeros((feat36.shape[1], S_NODE), dtype=np.float32)
        realn = cc["node_s"] >= 0
        f36[:, realn] = feat36[cc["node_s"][realn]].T

        per_core_inputs.append(dict(
            slotidx=slotidx_full, sendidx=sendidx_full,
            rhs18=rhs18.astype(NPBF16),
            mask64=mask64, feat36T=f36,
        ))

    meta["slot_col"] = slot_col
    meta["cores"] = cores
    return meta, per_core_inputs


def _fold_weights(w):
    """Host-side weight refactoring (see module docstring for the algebra)."""
    out = {}
    f32 = lambda a: np.ascontiguousarray(a, dtype=np.float32)
    bf = lambda a: np.ascontiguousarray(np.asarray(a, dtype=np.float32)).astype(NPBF16)
    D = w["hy_w1"].shape[0] // 3
    b2 = w["fx_b2"].astype(np.float32)          # x = x~ + fx_b2
    yb2 = w["fy_b2"].astype(np.float32)         # y = y~ + fy_b2

    out["hx_w1"] = f32(w["hx_w1"])              # [36, 64]
    out["hx_w2"] = f32(w["hx_w2"])
    out["hx_b1"] = f32(w["hx_b1"][:, None])
    # hx output goes straight into x~ state: subtract fx_b2
    hxb = np.zeros((P, 1), np.float32)
    hxb[H:, 0] = w["hx_b2"] - b2
    out["hxb2"] = hxb                            # [128,1], rows 64:128

    U = w["hy_w1"]
    hyAB = np.vstack([U[2 * D:3 * D] - U[0:D],   # vi = vc[src]
                      U[0:D] + U[D:2 * D]])      # vj = vc[dst]
    hyw = np.zeros((P, P), np.float32)
    hyw[0:2 * D, H:] = hyAB                      # z1 -> out rows 64:128
    hyw[H:, 0:H] = w["hy_w2"]                    # z2 -> out rows 0:64
    out["hyw_comb"] = bf(hyw)                    # [128, 128] bf16
    out["hyw2_hi"] = bf(np.vstack([np.zeros((H, H), np.float32), w["hy_w2"]]))
    hyb = np.zeros((P, 1), np.float32)
    hyb[0:H, 0] = w["hy_b2"] - yb2               # y~ = y - fy_b2
    hyb[H:, 0] = w["hy_b1"]
    out["hyb"] = hyb

    W1 = w["fx_w1"]
    V1 = w["fy_w1"]
    fxA = W1[64:128] + W1[0:64]                  # xj = x[src] (gathered)
    fxB = W1[128:192] - W1[0:64]                 # xi = x[dst] (own)
    fxC = W1[192:256]
    fyB = V1[128:192] - V1[0:64]                 # xj
    fyA = V1[0:64] + V1[64:128]                  # xi
    out["wAB"] = bf(np.vstack([np.hstack([fyB, fxA]),     # K rows 0:64: xj~
                               np.hstack([fyA, fxB])]))   # K rows 64:128: xi~
    out["fxC"] = bf(fxC)                         # [64, 64]
    w2d = np.zeros((P, P), np.float32)
    w2d[0:H, 0:H] = w["fy_w2"]
    w2d[H:, H:] = w["fx_w2"]
    out["w2diag"] = bf(w2d)                      # [128, 128]
    out["fxw2_hi"] = bf(np.vstack([np.zeros((H, H), np.float32), w["fx_w2"]]))
    b1p = np.zeros((P, 1), np.float32)
    b1p[0:H, 0] = w["fy_b1"] + (fyB.T + fyA.T) @ b2
    b1p[H:, 0] = w["fx_b1"] + (fxA.T + fxB.T) @ b2 + fxC.T @ yb2
    out["b1pair"] = b1p                          # [128,1]

    out["feta_w1_hi"] = f32(np.vstack([np.zeros((H, H), np.float32),
                                       w["feta_w1"]]))  # rows 64:128
    out["feta_w2"] = f32(w["feta_w2"])
    out["feta_w3"] = f32(w["feta_w3"])
    out["feta_b1e"] = f32((w["feta_b1"] + w["feta_w1"].T @ b2)[:, None])
    out["feta_b2"] = f32(w["feta_b2"][:, None])
    return out


_WDTYPES = dict(hx_w1=F32, hx_w2=F32, hx_b1=F32, hxb2=F32,
                hyw_comb=BF16, hyw2_hi=BF16, hyb=F32,
                wAB=BF16, fxC=BF16, w2diag=BF16, fxw2_hi=BF16,
                b1pair=F32,
                feta_w1_hi=F32, feta_w2=F32, feta_w3=F32,
                feta_b1e=F32, feta_b2=F32)


def _build(meta, wshapes, LOOP):
    S_NODE, S_E, NBLK = meta["S_NODE"], meta["S_E"], meta["NBLK"]
    NMASK, R = meta["NMASK"], meta["R"]
    RJ = R // P
    tiles = meta["tiles"]
    mask_index = meta["mask_index"]
    K18 = 2 * meta["D"]

    nc = bacc.Bacc("TRN2", target_bir_lowering=False, debug=False,
                   num_devices=1 if SIM_SINGLE else NCORE,
                   num_swdge_queues=4)

    # ---- inputs ----
    din = {}
    for name, shp in wshapes.items():
        din[name] = nc.dram_tensor(name, list(shp), _WDTYPES[name],
                                   kind="ExternalInput")
    feat36T = nc.dram_tensor("feat36T", [wshapes["hx_w1"][0], S_NODE], F32,
                             kind="ExternalInput")
    rhs18 = nc.dram_tensor("rhs18", [K18, S_E], BF16, kind="ExternalInput")
    slotidx = nc.dram_tensor("slotidx", [P, S_E // 16], I16, kind="ExternalInput")
    sendidx = nc.dram_tensor("sendidx", [P, NCORE * R // 16], I16,
                             kind="ExternalInput")
    mask64 = nc.dram_tensor("mask64", [H, NMASK * P], F32, kind="ExternalInput")

    outslots = nc.dram_tensor("outslots", [S_NODE, 1], F32, kind="ExternalOutput")

    # ---- internal DRAM ----
    yT = nc.dram_tensor("yT", [H, S_E], BF16)
    myslice = nc.dram_tensor("myslice", [S_NODE, P], BF16)
    sendbuf = nc.dram_tensor("sendbuf", [NCORE * R, P], BF16)
    recvbuf = nc.dram_tensor("recvbuf", [NCORE * R, P], BF16)

    myslice_pview = myslice.ap().rearrange("(b p) f -> p b f", p=P)
    outslots_pview = outslots.ap().rearrange("(b p) o -> p b o", p=P)

    ACT = mybir.ActivationFunctionType
    ALU = mybir.AluOpType

    with tile.TileContext(nc) as tc:
        with (
            tc.tile_pool(name="persist", bufs=1) as pp,
            tc.tile_pool(name="work", bufs=3) as wp,
            tc.tile_pool(name="hpool", bufs=3) as hq,
            tc.tile_pool(name="sendp", bufs=2) as sp,
            tc.tile_pool(name="callbuf", bufs=8) as cbp,
            tc.tile_pool(name="pza", bufs=4, space="PSUM") as pza,
            tc.tile_pool(name="pzb", bufs=2, space="PSUM") as pzb,
            tc.tile_pool(name="ptrp", bufs=2, space="PSUM") as ptrp,
        ):
            # ---- persistent tiles ----
            identb = pp.tile([P, P], BF16, tag="identb")
            make_identity(nc, identb[:])
            W = {}
            for name, shp in wshapes.items():
                t = pp.tile(list(shp), _WDTYPES[name], tag=f"w_{name}")
                nc.sync.dma_start(out=t[:], in_=din[name][:, :])
                W[name] = t
            xt = pp.tile([P, S_NODE], F32, tag="xt")      # rows 64:128 = x~
            xbf = pp.tile([P, S_NODE], BF16, tag="xbf")   # rows 64:128 = bf16(x~)
            staging = pp.tile([P, NBLK * H], BF16, tag="staging")
            staging2 = pp.tile([P, NBLK], F32, tag="staging2")
            sidx = pp.tile([P, S_E // 16], I16, tag="sidx")
            nc.sync.dma_start(out=sidx[:], in_=slotidx[:, :])
            kidx = pp.tile([P, NCORE * R // 16], I16, tag="kidx")
            nc.sync.dma_start(out=kidx[:], in_=sendidx[:, :])
            msk = pp.tile([P, NMASK * P], F32, tag="msk")
            nc.sync.dma_start(out=msk[H:P, :], in_=mask64[:, :])

            # zero myslice's pad half once (gathered but never consumed)
            nc.vector.memset(staging[:, :], 0.0)
            nc.sync.dma_start(
                out=myslice_pview[:, :, H:P],
                in_=staging[:].rearrange("p (b f) -> p b f", b=NBLK))

            evac_ct = [0]

            def evac(dst_ap, src_ap):
                if evac_ct[0] % 2 == 0:
                    nc.scalar.copy(out=dst_ap, in_=src_ap)
                else:
                    nc.vector.tensor_copy(out=dst_ap, in_=src_ap)
                evac_ct[0] += 1

            # ---------- x~ scatter-max consume (with mask on flagged subtiles) ----------
            def consume(col0, w, sbase, w2p):
                j = 0
                while j < w // P:
                    gsub = (sbase + j * P) // P
                    if gsub in mask_index:
                        mi = mask_index[gsub]
                        tmp = wp.tile([P, P], F32, tag="mtmp")
                        nc.vector.tensor_tensor(
                            out=tmp[H:P, :], in0=w2p[H:P, j * P:(j + 1) * P],
                            in1=msk[H:P, mi * P:(mi + 1) * P], op=ALU.add)
                        nc.vector.tensor_tensor(
                            out=xt[H:P, col0 + j * P:col0 + (j + 1) * P],
                            in0=xt[H:P, col0 + j * P:col0 + (j + 1) * P],
                            in1=tmp[H:P, :], op=ALU.max)
                        j += 1
                    else:
                        j2 = j
                        while j2 < w // P and ((sbase + j2 * P) // P) not in mask_index:
                            j2 += 1
                        nc.vector.tensor_tensor(
                            out=xt[H:P, col0 + j * P:col0 + j2 * P],
                            in0=xt[H:P, col0 + j * P:col0 + j2 * P],
                            in1=w2p[H:P, j * P:j2 * P], op=ALU.max)
                        j = j2

            # ---------- readback: x~ -> xbf -> myslice (transposed bf16) ----------
            def readback():
                nc.vector.tensor_copy(out=xbf[H:P, :], in_=xt[H:P, :])
                for b in range(NBLK):
                    ps = ptrp.tile([P, TILE_W], F32, tag="ptr")
                    psb = ps[:].bitcast(BF16)[:, 0:H]
                    nc.tensor.transpose(
                        out=psb, in_=xbf[H:P, b * P:(b + 1) * P],
                        identity=identb[H:P, H:P])
                    evac(staging[:, b * H:(b + 1) * H], psb)
                nc.sync.dma_start(
                    out=myslice_pview[:, :, 0:H],
                    in_=staging[:].rearrange("p (b f) -> p b f", b=NBLK))

            # queue_num must track the global Pool-DMA instruction order:
            # tile_sem_assignment rotates DMASW sem lanes per instruction and
            # each lane is serviced by the matching SWDGE queue.
            gq = [0]

            def next_q():
                q = gq[0] % 4
                gq[0] += 1
                return q

            # ---------- exchange: myslice -> sendbuf -> A2A -> recvbuf ----------
            def exchange():
                for dd in range(NCORE):
                    st = sp.tile([P, RJ, P], BF16, tag="sendt")
                    off = 0
                    while off < R:
                        n = min(GC, R - off)
                        nc.gpsimd.dma_gather(
                            out_ap=st[:, off // P:(off + n) // P, :],
                            in_ap=myslice[:, :],
                            idxs_ap=kidx[:, (dd * R + off) // 16:(dd * R + off + n) // 16],
                            num_idxs=n, num_idxs_reg=n, elem_size=P,
                            queue_num=next_q())
                        off += n
                    dv = sendbuf.ap()[dd * R:(dd + 1) * R, :].rearrange(
                        "(p j) f -> p (j f)", p=P)
                    nc.sync.dma_start(out=dv, in_=st[:].rearrange("p j f -> p (j f)"))
                if SIM_SINGLE:
                    nc.sync.dma_start(out=recvbuf.ap().rearrange(
                        "(p a) f -> p (a f)", p=P),
                        in_=sendbuf.ap().rearrange("(p a) f -> p (a f)", p=P))
                else:
                    nc.gpsimd.collective_compute(
                        "AllToAll", ALU.bypass,
                        replica_groups=[list(range(NCORE))],
                        ins=[sendbuf.ap().bitcast(F32)],
                        outs=[recvbuf.ap().bitcast(F32)])

            # ---------- slot gather: recvbuf -> callbuf chunks (row-major) ----------
            def slot_gather():
                bufs = []
                base = 0
                while base < S_E:
                    n = min(GC, S_E - base)
                    st = cbp.tile([P, GC // P, P], BF16, tag="cb")
                    nc.gpsimd.dma_gather(
                        out_ap=st[:, : n // P, :], in_ap=recvbuf[:, :],
                        idxs_ap=sidx[:, base // 16:(base + n) // 16],
                        num_idxs=n, num_idxs_reg=n, elem_size=P,
                        queue_num=next_q())
                    bufs.append(st)
                    base += n
                return bufs

            # ---------- per-tile stacked rhs: rows 0:64 = xj~^T (PE-transposed
            # gathered blocks), rows 64:128 = xi~ (aligned copy of xbf) ----------
            def make_stk(bufs, col0, sbase, w):
                stk = wp.tile([P, TILE_W], BF16, tag="stk")
                for j in range(w // P):
                    s = sbase + j * P
                    g = bufs[s // GC][:, (s % GC) // P, 0:H]
                    ps = ptrp.tile([P, TILE_W], F32, tag="ptr")
                    psb = ps[:].bitcast(BF16)[0:H, 0:P]
                    nc.tensor.transpose(out=psb, in_=g, identity=identb[:, :])
                    evac(stk[0:H, j * P:(j + 1) * P], psb)
                nc.vector.tensor_copy(out=stk[H:P, :w],
                                      in_=xbf[H:P, col0:col0 + w])
                return stk

            # ---------- fused phase: fy(k) then fx(k), sharing gathered x ----------
            def fused_phase(k, bufs):
                KSUB = int(os.environ.get("KSUB", "9"))
                with_fy = k > 0
                write_y = k < LOOP - 1
                hp_cur = None     # [h1y(t); h1x(t-1)]
                pend = None       # (col0, w, sbase) of tile t-1 awaiting z2x
                for ti, (r, col0, w, sbase) in enumerate(tiles):
                    if ti >= int(os.environ.get("KTILES", "9999")):
                        break
                    stk = make_stk(bufs, col0, sbase, w)
                    if KSUB < 1:
                        continue
                    z = pza.tile([P, TILE_W], F32, tag="z")
                    nc.tensor.matmul(z[:, :w], W["wAB"][:], stk[:, :w],
                                     start=True, stop=True)
                    if KSUB < 2:
                        continue
                    yt = wp.tile([H, TILE_W], BF16, tag="yt")
                    nc.sync.dma_start(out=yt[:, :w], in_=yT[:, sbase:sbase + w])
                    if with_fy:
                        if hp_cur is None:
                            hp_cur = hq.tile([P, TILE_W], BF16, tag="hp")
                            nc.vector.memset(hp_cur[H:P, :], 0.0)
                        nc.scalar.activation(out=hp_cur[0:H, :w], in_=z[0:H, :w],
                                             func=ACT.Relu,
                                             bias=W["b1pair"][0:H, 0:1])
                        wz = max(w, pend[1]) if pend is not None else w
                        if pend is not None and pend[1] > w:
                            # h1y(t) gap: z2 streams wz cols, relu wrote only w
                            nc.vector.memset(hp_cur[0:H, w:pend[1]], 0.0)
                        if pend is not None and w > pend[1]:
                            # h1x(t-1) gap: written only to pend[1]
                            nc.vector.memset(hp_cur[H:P, pend[1]:w], 0.0)
                        w2p = pzb.tile([P, TILE_W], F32, tag="w2p")
                        nc.tensor.matmul(w2p[:, :wz], W["w2diag"][:],
                                         hp_cur[:, :wz], start=True, stop=True)
                        # y~ = max(y~, z2y(t))
                        nc.vector.tensor_tensor(out=yt[:, :w], in0=yt[:, :w],
                                                in1=w2p[0:H, :w], op=ALU.max)
                        if write_y:
                            nc.sync.dma_start(out=yT[:, sbase:sbase + w],
                                              in_=yt[:, :w])
                        if pend is not None:
                            consume(pend[0], pend[1], pend[2], w2p)
                        pend = (col0, w, sbase)
                    nc.tensor.matmul(z[H:P, :w], W["fxC"][:], yt[:, :w],
                                     start=False, stop=True, skip_group_check=True)
                    if KSUB < 3:
                        continue
                    hp_next = hq.tile([P, TILE_W], BF16, tag="hp")
                    nc.scalar.activation(out=hp_next[H:P, :w], in_=z[H:P, :w],
                                         func=ACT.Relu, bias=W["b1pair"][H:P, 0:1])
                    if KSUB < 4:
                        continue
                    if not with_fy:
                        w2p = pzb.tile([P, TILE_W], F32, tag="w2p")
                        nc.tensor.matmul(w2p[H:P, :w], W["fxw2_hi"][H:P, :],
                                         hp_next[H:P, :w], start=True, stop=True)
                        consume(col0, w, sbase, w2p)
                    hp_cur = hp_next
                if with_fy:
                    # flush: z2x of the last tile
                    lc, lw, lsb = pend
                    w2p = pzb.tile([P, TILE_W], F32, tag="w2p")
                    nc.tensor.matmul(w2p[H:P, :lw], W["fxw2_hi"][H:P, :],
                                     hp_cur[H:P, :lw], start=True, stop=True)
                    consume(lc, lw, lsb, w2p)

            # ---------- init: hx (f32) ----------
            K36 = wshapes["hx_w1"][0]
            off = 0
            while off < S_NODE:
                w = min(TILE_W, S_NODE - off)
                ft = wp.tile([K36, TILE_W], F32, tag="ft")
                nc.sync.dma_start(out=ft[:, :w], in_=feat36T[:, off:off + w])
                z1 = pza.tile([P, TILE_W], F32, tag="z")
                nc.tensor.matmul(z1[0:H, :w], W["hx_w1"][:], ft[:, :w],
                                 start=True, stop=True)
                h1 = wp.tile([H, TILE_W], F32, tag="h1")
                nc.scalar.activation(out=h1[:, :w], in_=z1[0:H, :w],
                                     func=ACT.Relu, bias=W["hx_b1"][:, 0:1])
                z2 = pza.tile([P, TILE_W], F32, tag="z")
                nc.tensor.matmul(z2[H:P, :w], W["hx_w2"][:], h1[:, :w],
                                 start=True, stop=True)
                nc.scalar.activation(out=xt[H:P, off:off + w], in_=z2[H:P, :w],
                                     func=ACT.Identity, bias=W["hxb2"][H:P, 0:1])
                off += w

            # ---------- init: hy (bf16, one pipelined pass per tile) ----------
            # pass t: z1(t) (rows 64:128) from rt[0:18]=r18(t);
            #         z2(t-1) (rows 0:64) from rt[64:128]=h1y(t-1)
            rt_cur = wp.tile([P, TILE_W], BF16, tag="rt")
            nc.vector.memset(rt_cur[:, :], 0.0)
            nc.sync.dma_start(out=rt_cur[0:K18, :tiles[0][2]],
                              in_=rhs18[:, 0:tiles[0][2]])
            pw = 0
            psb_prev = 0
            for ti, (r, col0, w, sbase) in enumerate(tiles):
                wz = max(w, pw)
                zi = pza.tile([P, TILE_W], F32, tag="z")
                nc.tensor.matmul(zi[:, :wz], W["hyw_comb"][:], rt_cur[:, :wz],
                                 start=True, stop=True)
                if ti > 0:
                    yt0 = wp.tile([H, TILE_W], BF16, tag="yt")
                    nc.scalar.activation(out=yt0[:, :pw], in_=zi[0:H, :pw],
                                         func=ACT.Identity, bias=W["hyb"][0:H, 0:1])
                    nc.sync.dma_start(out=yT[:, psb_prev:psb_prev + pw],
                                      in_=yt0[:, :pw])
                if ti + 1 < len(tiles):
                    nw = tiles[ti + 1][2]
                    rt_next = wp.tile([P, TILE_W], BF16, tag="rt")
                    nc.vector.memset(rt_next[:, :], 0.0)
                    nc.sync.dma_start(
                        out=rt_next[0:K18, :nw],
                        in_=rhs18[:, tiles[ti + 1][3]:tiles[ti + 1][3] + nw])
                    nc.scalar.activation(out=rt_next[H:P, :w], in_=zi[H:P, :w],
                                         func=ACT.Relu, bias=W["hyb"][H:P, 0:1])
                    rt_cur = rt_next
                else:
                    # flush: z2 of the last tile via hy_w2-only pass
                    hlast = wp.tile([P, TILE_W], BF16, tag="rt")
                    nc.scalar.activation(out=hlast[H:P, :w], in_=zi[H:P, :w],
                                         func=ACT.Relu, bias=W["hyb"][H:P, 0:1])
                    zf = pza.tile([P, TILE_W], F32, tag="z")
                    nc.tensor.matmul(zf[0:H, :w], W["hyw2_hi"][H:P, :],
                                     hlast[H:P, :w], start=True, stop=True)
                    ytf = wp.tile([H, TILE_W], BF16, tag="yt")
                    nc.scalar.activation(out=ytf[:, :w], in_=zf[0:H, :w],
                                         func=ACT.Identity, bias=W["hyb"][0:H, 0:1])
                    nc.sync.dma_start(out=yT[:, sbase:sbase + w], in_=ytf[:, :w])
                pw = w
                psb_prev = sbase

            # ---------- initial exchange of x0 ----------
            KSTAGE = int(os.environ.get("KSTAGE", "0"))
            readback()
            exchange()

            # ---------- iterations ----------
            if KSTAGE != 1:
                for k in range(LOOP):
                    bufs = slot_gather()
                    if KSTAGE == 2:
                        break
                    fused_phase(k, bufs)
                    if KSTAGE == 3:
                        break
                    if k < LOOP - 1:
                        readback()
                        exchange()

            # ---------- final MLP (f32) ----------
            off = 0
            while off < S_NODE:
                w = min(TILE_W, S_NODE - off)
                z1 = pza.tile([P, TILE_W], F32, tag="z")
                nc.tensor.matmul(z1[0:H, :w], W["feta_w1_hi"][H:P, :],
                                 xt[H:P, off:off + w], start=True, stop=True)
                h1 = wp.tile([H, TILE_W], F32, tag="h1")
                nc.scalar.activation(out=h1[:, :w], in_=z1[0:H, :w],
                                     func=ACT.Relu, bias=W["feta_b1e"][:, 0:1])
                z2 = pza.tile([P, TILE_W], F32, tag="z")
                nc.tensor.matmul(z2[0:H, :w], W["feta_w2"][:], h1[:, :w],
                                 start=True, stop=True)
                h2 = wp.tile([H, TILE_W], F32, tag="h2")
                nc.scalar.activation(out=h2[:, :w], in_=z2[0:H, :w],
                                     func=ACT.Relu, bias=W["feta_b2"][:, 0:1])
                for j in range(w // P):
                    b = (off + j * P) // P
                    ps = pzb.tile([P, TILE_W], F32, tag="w2p")
                    nc.tensor.matmul(ps[:, 0:1], h2[:, j * P:(j + 1) * P],
                                     W["feta_w3"][:], start=True, stop=True)
                    evac(staging2[:, b:b + 1], ps[:, 0:1])
                off += w
            nc.sync.dma_start(
                out=outslots_pview,
                in_=staging2[:].rearrange("p (b o) -> p b o", b=NBLK))

    _log(f"built program: {S_E=} {len(tiles)=} masks={NMASK} R={R}")
    nc.compile()
    _log("compiled")
    return nc


def kernel(**inputs):
    global LAST_EXEC_NS, LAST_TRACE
    v = np.asarray(inputs["v"], dtype=np.float32)
    labels = np.asarray(inputs["labels"], dtype=np.float32)
    edge_index = np.asarray(inputs["edge_index"]).astype(np.int64)
    LOOP = int(np.asarray(inputs["loop"]))

    import hashlib
    ck = hashlib.sha1(edge_index.tobytes()).hexdigest() + f"_{LOOP}_{v.shape}"
    if ck in _BUILD_CACHE:
        meta, pci, nc = _BUILD_CACHE[ck]
    else:
        meta, pci, nc = None, None, None
    if meta is None:
        meta, pci = _preprocess(v, labels, edge_index)
    wf = _fold_weights({k: np.asarray(val, dtype=np.float32)
                        for k, val in inputs.items()
                        if k not in ("v", "labels", "edge_index", "loop")})
    wnames = list(_WDTYPES.keys())
    wshapes = {n: wf[n].shape for n in wnames}

    if nc is None:
        nc = _build(meta, wshapes, LOOP)
        _BUILD_CACHE[ck] = (meta, pci, nc)

    in_maps = []
    for c in range(NCORE):
        m = {n: wf[n] for n in wnames}
        m["feat36T"] = pci[c]["feat36T"]
        m["rhs18"] = pci[c]["rhs18"]
        m["slotidx"] = pci[c]["slotidx"]
        m["sendidx"] = pci[c]["sendidx"]
        m["mask64"] = pci[c]["mask64"]
        in_maps.append(m)

    res = run_bass_kernel_spmd(nc, in_maps, core_ids=list(range(NCORE)),
                               tmpdir=os.environ.get("BASS_TMPDIR") or None)
    LAST_EXEC_NS = res.exec_time_ns
    LAST_TRACE = res.instructions_and_trace

    N = meta["N"]
    out = np.zeros((N, 1), dtype=np.float32)
    for c in range(NCORE):
        cc = meta["cores"][c]
        slots = cc["slot_of_local"]
        vals = res.results[c]["outslots"][:, 0]
        out[cc["lo"]:cc["lo"] + cc["nloc"], 0] = vals[slots]
    return out


# revision 40
# speedup vs baseline: 2.0577x; 1.2236x over previous
"""Explorer GNN message-passing kernel for 8 TRN2 NeuronCores (Bass/Tile).

Strategy (node-sharded, edge-local), v2:
  - Nodes split contiguously across 8 cores. Each core owns every edge whose
    dst falls in its range, so segment-max is core-local. Owned nodes are
    permuted by ascending in-degree into "slots" (blocks of 128); edges laid
    out in (round, block, partition) order so one round-tile of messages
    max-combines into a contiguous column range of the feature-major node
    state with a single DVE max.
  - Node state is kept as x~ = x - fx_b2 on SBUF partitions 64:128 for the
    whole kernel; all biases are folded so scatter-max and the y-update need
    no bias ops at all.
  - x rows are exchanged between cores as bf16 256-byte rows; the per-edge
    x[src] gather uses dma_gather(transpose=True), which lands the rows
    FEATURE-MAJOR in SBUF - no PE transposes in the inner loop, and all
    inner-loop matmuls run in bf16 (2x PE rate):
      z1  = [fyB|fxA]^T xj~  (+)  [fyA|fxB]^T xi~   (two K=64->M=128 passes)
      z1x += fxC^T y~                                (K=64 quadrant pass)
      z2  = diag(fy_w2, fx_w2)^T [h1y(t); h1x(t-1)]  (one K=128->M=128 pass,
            software-pipelined across tiles)
  - The edge-init MLP (hy) is similarly pipelined into ONE matmul per tile
    with lhsT [[0,hyAB],[hy_w2,0]].
"""

import os
import sys
import numpy as np
import ml_dtypes

import concourse.bass as bass
import concourse.mybir as mybir
import concourse.bacc as bacc
import concourse.tile as tile
from concourse.bass_utils import run_bass_kernel_spmd
from concourse.masks import make_identity

NCORE = 8
P = 128
H = 64
TILE_W = 512
GC = 1024   # max indices per dma_gather call (ring limit < 2048)
# NOTE: dma_gather(transpose=True) silently corrupts data beyond ~16
# back-to-back calls (HW-verified); gathered rows are transposed on the
# PE instead (bf16: 1 cyc/row).
NEG = -1.0e30
F32 = mybir.dt.float32
BF16 = mybir.dt.bfloat16
I16 = mybir.dt.int16
NPBF16 = ml_dtypes.bfloat16

LAST_EXEC_NS = None
LAST_TRACE = None
_BUILD_CACHE = {}
SIM_SINGLE = False  # build single-core variant (collective -> DMA)


def _log(msg):
    print(f"[kernel] {msg}", file=sys.stderr, flush=True)


def _wrap16(ids, ncols):
    out = np.zeros((16, ncols), dtype=np.int16)
    n = len(ids)
    out[np.arange(n) % 16, np.arange(n) // 16] = ids.astype(np.int16)
    return out


def _wrap16_chunks(ids, chunk):
    n = len(ids)
    out = np.zeros((16, n // 16), dtype=np.int16)
    off = 0
    while off < n:
        m = min(chunk, n - off)
        out[:, off // 16:(off + m) // 16] = _wrap16(ids[off:off + m], m // 16)
        off += m
    return out


def _preprocess(v, labels, edge_index):
    N, C = v.shape
    D = C + 2
    E = edge_index.shape[1]
    NODE_LOC = (N + NCORE - 1) // NCORE
    NBLK = (NODE_LOC + P - 1) // P
    S_NODE = NBLK * P

    vc = np.concatenate([v, labels], axis=1).astype(np.float32)  # [N, D]
    gi = int(np.argmax(labels[:, 1]))
    goal = vc[gi]
    d = vc - goal
    feat36 = np.concatenate(
        [vc, np.broadcast_to(goal, vc.shape), d, d * d], axis=1
    ).astype(np.float32)  # [N, 4D]

    src = edge_index[0].astype(np.int64)
    dst = edge_index[1].astype(np.int64)
    owner = dst // NODE_LOC

    cores = []
    for c in range(NCORE):
        lo, hi = c * NODE_LOC, min((c + 1) * NODE_LOC, N)
        nloc = hi - lo
        eids = np.where(owner == c)[0]
        dl = dst[eids] - lo
        deg = np.bincount(dl, minlength=nloc)
        order = np.argsort(deg, kind="stable")  # ascending degree
        slot_of_local = np.empty(nloc, dtype=np.int64)
        slot_of_local[order] = np.arange(nloc)
        es = eids[np.argsort(dl, kind="stable")]
        rp = np.zeros(nloc + 1, dtype=np.int64)
        rp[1:] = np.cumsum(deg)
        deg_s = np.zeros(S_NODE, dtype=np.int64)
        deg_s[:nloc] = deg[order]
        node_s = np.full(S_NODE, -1, dtype=np.int64)
        node_s[:nloc] = order + lo
        rp_s = np.zeros(S_NODE, dtype=np.int64)
        rp_s[:nloc] = rp[order]
        Rb = np.zeros(NBLK, dtype=np.int64)
        for b in range(NBLK):
            Rb[b] = deg_s[b * P:(b + 1) * P].max()
        cores.append(
            dict(lo=lo, nloc=nloc, deg_s=deg_s, node_s=node_s, rp_s=rp_s,
                 es=es, Rb=Rb, slot_of_local=slot_of_local)
        )

    Rb = np.max(np.stack([cc["Rb"] for cc in cores]), axis=0)  # [NBLK]
    assert np.all(np.diff(Rb) >= 0), "Rb must be nondecreasing"
    maxR = int(Rb.max())

    # tile structure (uniform across cores): per round, chunk the block-suffix
    tiles = []  # (r, col0, w, sbase)
    sbase = 0
    for r in range(maxR):
        b_r = int(np.searchsorted(Rb, r + 1))
        col0 = b_r * P
        wtot = (NBLK - b_r) * P
        off = 0
        while off < wtot:
            w = min(TILE_W, wtot - off)
            tiles.append((r, col0 + off, w, sbase + off))
            off += w
        sbase += wtot
    S_E = sbase
    n_sub = S_E // P

    slot_edge = np.full((NCORE, S_E), -1, dtype=np.int64)
    slot_col = np.empty(S_E, dtype=np.int64)
    spos = 0
    for r in range(maxR):
        b_r = int(np.searchsorted(Rb, r + 1))
        cols = np.arange(b_r * P, NBLK * P)
        n_s = len(cols)
        slot_col[spos:spos + n_s] = cols
        for c in range(NCORE):
            cc = cores[c]
            degc = cc["deg_s"][cols]
            rpc = cc["rp_s"][cols]
            has = degc > r
            dup = (~has) & (degc > 0)
            e = np.full(n_s, -1, dtype=np.int64)
            e[has] = cc["es"][rpc[has] + r]
            e[dup] = cc["es"][rpc[dup]]
            slot_edge[c, spos:spos + n_s] = e
        spos += n_s
    assert spos == S_E

    # masked subtiles: any core has a pad slot (-1 edge) on a REAL node there
    sub_masked = np.zeros(n_sub, dtype=bool)
    for si in range(n_sub):
        cols = slot_col[si * P: si * P + P]
        for c in range(NCORE):
            cc = cores[c]
            e = slot_edge[c, si * P: si * P + P]
            real = cc["node_s"][cols] >= 0
            if np.any((e < 0) & real):
                sub_masked[si] = True
                break
    masked_ids = np.where(sub_masked)[0]
    mask_index = {int(s): i for i, s in enumerate(masked_ids)}
    NMASK = max(1, len(masked_ids))

    # request lists: req[c][d] = sorted unique src nodes of core c's edges owned by d
    req = [[None] * NCORE for _ in range(NCORE)]
    maxlen = 0
    for c in range(NCORE):
        e = slot_edge[c]
        srcs = np.unique(src[e[e >= 0]])
        bounds = np.searchsorted(srcs, np.arange(1, NCORE) * NODE_LOC)
        parts = np.split(srcs, bounds)
        for dd in range(NCORE):
            req[c][dd] = parts[dd]
            maxlen = max(maxlen, len(parts[dd]))
    R = ((maxlen + P - 1) // P) * P
    RJ = R // P

    per_core_inputs = []
    meta = dict(N=N, C=C, D=D, E=E, NODE_LOC=NODE_LOC, NBLK=NBLK,
                S_NODE=S_NODE, S_E=S_E, maxR=maxR, tiles=tiles,
                masked_ids=masked_ids.tolist(), mask_index=mask_index,
                NMASK=NMASK, R=R, n_sub=n_sub)

    for c in range(NCORE):
        cc = cores[c]
        # receive-position map: node -> recvbuf row
        posmap = np.zeros(N, dtype=np.int64)
        for dd in range(NCORE):
            lst = req[c][dd]
            i = np.arange(len(lst))
            kc = i // GC
            i2 = i % GC
            posmap[lst] = dd * R + (i2 % P) * RJ + kc * (GC // P) + i2 // P
        e = slot_edge[c]
        has_e = e >= 0
        srcn = np.where(has_e, src[np.clip(e, 0, None)], 0)
        srcpos = np.where(has_e, posmap[srcn], 0)
        ncols16 = S_E // 16
        slotidx = np.zeros((16, ncols16), dtype=np.int16)
        base = 0
        while base < S_E:
            n = min(GC, S_E - base)
            w = _wrap16(srcpos[base:base + n], n // 16)
            slotidx[:, base // 16: (base + n) // 16] = w
            base += n
        slotidx_full = np.tile(slotidx, (8, 1))  # [128, S_E/16]

        sendidx = np.zeros((16, NCORE * R // 16), dtype=np.int16)
        for dd in range(NCORE):
            lst = req[dd][c]
            rows = cc["slot_of_local"][lst - cc["lo"]]
            rows = np.concatenate([rows, np.zeros(R - len(rows), dtype=np.int64)])
            sendidx[:, dd * (R // 16): (dd + 1) * (R // 16)] = _wrap16_chunks(rows, GC)
        sendidx_full = np.tile(sendidx, (8, 1))

        # rhs18 for hy (bf16): rows 0:9 = vc[src(e)], rows 9:18 = vc[dst(e)]
        rhs18 = np.zeros((2 * D, S_E), dtype=np.float32)
        dstn = np.where(has_e, dst[np.clip(e, 0, None)], 0)
        rhs18[:D, has_e] = vc[srcn[has_e]].T
        rhs18[D:, has_e] = vc[dstn[has_e]].T

        # mask data [64, NMASK*128]
        mask64 = np.zeros((H, NMASK * P), dtype=np.float32)
        for i, si in enumerate(masked_ids):
            cols = slot_col[si * P: si * P + P]
            ee = slot_edge[c, si * P: si * P + P]
            real = cc["node_s"][cols] >= 0
            dead = (ee < 0) & real
            mask64[:, i * P: (i + 1) * P][:, dead] = NEG

        # node-init features, slot order, transposed
        f36 = np.zeros((feat36.shape[1], S_NODE), dtype=np.float32)
        realn = cc["node_s"] >= 0
        f36[:, realn] = feat36[cc["node_s"][realn]].T

        per_core_inputs.append(dict(
            slotidx=slotidx_full, sendidx=sendidx_full,
            rhs18=rhs18.astype(NPBF16),
            mask64=mask64, feat36T=f36,
        ))

    meta["slot_col"] = slot_col
    meta["cores"] = cores
    return meta, per_core_inputs


def _fold_weights(w):
    """Host-side weight refactoring (see module docstring for the algebra)."""
    out = {}
    f32 = lambda a: np.ascontiguousarray(a, dtype=np.float32)
    bf = lambda a: np.ascontiguousarray(np.asarray(a, dtype=np.float32)).astype(NPBF16)
    D = w["hy_w1"].shape[0] // 3
    b2 = w["fx_b2"].astype(np.float32)          # x = x~ + fx_b2
    yb2 = w["fy_b2"].astype(np.float32)         # y = y~ + fy_b2

    out["hx_w1"] = f32(w["hx_w1"])              # [36, 64]
    out["hx_w2"] = f32(w["hx_w2"])
    out["hx_b1"] = f32(w["hx_b1"][:, None])
    # hx output goes straight into x~ state: subtract fx_b2
    hxb = np.zeros((P, 1), np.float32)
    hxb[H:, 0] = w["hx_b2"] - b2
    out["hxb2"] = hxb                            # [128,1], rows 64:128

    U = w["hy_w1"]
    hyAB = np.vstack([U[2 * D:3 * D] - U[0:D],   # vi = vc[src]
                      U[0:D] + U[D:2 * D]])      # vj = vc[dst]
    hyw = np.zeros((P, P), np.float32)
    hyw[0:2 * D, H:] = hyAB                      # z1 -> out rows 64:128
    hyw[H:, 0:H] = w["hy_w2"]                    # z2 -> out rows 0:64
    out["hyw_comb"] = bf(hyw)                    # [128, 128] bf16
    out["hyw2_hi"] = bf(np.vstack([np.zeros((H, H), np.float32), w["hy_w2"]]))
    hyb = np.zeros((P, 1), np.float32)
    hyb[0:H, 0] = w["hy_b2"] - yb2               # y~ = y - fy_b2
    hyb[H:, 0] = w["hy_b1"]
    out["hyb"] = hyb

    W1 = w["fx_w1"]
    V1 = w["fy_w1"]
    fxA = W1[64:128] + W1[0:64]                  # xj = x[src] (gathered)
    fxB = W1[128:192] - W1[0:64]                 # xi = x[dst] (own)
    fxC = W1[192:256]
    fyB = V1[128:192] - V1[0:64]                 # xj
    fyA = V1[0:64] + V1[64:128]                  # xi
    out["wAB"] = bf(np.vstack([np.hstack([fyB, fxA]),     # K rows 0:64: xj~
                               np.hstack([fyA, fxB])]))   # K rows 64:128: xi~
    out["fxC"] = bf(fxC)                         # [64, 64]
    w2d = np.zeros((P, P), np.float32)
    w2d[0:H, 0:H] = w["fy_w2"]
    w2d[H:, H:] = w["fx_w2"]
    out["w2diag"] = bf(w2d)                      # [128, 128]
    out["fxw2_hi"] = bf(np.vstack([np.zeros((H, H), np.float32), w["fx_w2"]]))
    b1p = np.zeros((P, 1), np.float32)
    b1p[0:H, 0] = w["fy_b1"] + (fyB.T + fyA.T) @ b2
    b1p[H:, 0] = w["fx_b1"] + (fxA.T + fxB.T) @ b2 + fxC.T @ yb2
    out["b1pair"] = b1p                          # [128,1]

    out["feta_w1_hi"] = f32(np.vstack([np.zeros((H, H), np.float32),
                                       w["feta_w1"]]))  # rows 64:128
    out["feta_w2"] = f32(w["feta_w2"])
    out["feta_w3"] = f32(w["feta_w3"])
    out["feta_b1e"] = f32((w["feta_b1"] + w["feta_w1"].T @ b2)[:, None])
    out["feta_b2"] = f32(w["feta_b2"][:, None])
    return out


_WDTYPES = dict(hx_w1=F32, hx_w2=F32, hx_b1=F32, hxb2=F32,
                hyw_comb=BF16, hyw2_hi=BF16, hyb=F32,
                wAB=BF16, fxC=BF16, w2diag=BF16, fxw2_hi=BF16,
                b1pair=F32,
                feta_w1_hi=F32, feta_w2=F32, feta_w3=F32,
                feta_b1e=F32, feta_b2=F32)


def _build(meta, wshapes, LOOP):
    S_NODE, S_E, NBLK = meta["S_NODE"], meta["S_E"], meta["NBLK"]
    NMASK, R = meta["NMASK"], meta["R"]
    RJ = R // P
    tiles = meta["tiles"]
    mask_index = meta["mask_index"]
    K18 = 2 * meta["D"]

    nc = bacc.Bacc("TRN2", target_bir_lowering=False, debug=False,
                   num_devices=1 if SIM_SINGLE else NCORE,
                   num_swdge_queues=4)

    # ---- inputs ----
    din = {}
    for name, shp in wshapes.items():
        din[name] = nc.dram_tensor(name, list(shp), _WDTYPES[name],
                                   kind="ExternalInput")
    feat36T = nc.dram_tensor("feat36T", [wshapes["hx_w1"][0], S_NODE], F32,
                             kind="ExternalInput")
    rhs18 = nc.dram_tensor("rhs18", [K18, S_E], BF16, kind="ExternalInput")
    slotidx = nc.dram_tensor("slotidx", [P, S_E // 16], I16, kind="ExternalInput")
    sendidx = nc.dram_tensor("sendidx", [P, NCORE * R // 16], I16,
                             kind="ExternalInput")
    mask64 = nc.dram_tensor("mask64", [H, NMASK * P], F32, kind="ExternalInput")

    outslots = nc.dram_tensor("outslots", [S_NODE, 1], F32, kind="ExternalOutput")

    # ---- internal DRAM ----
    yT = nc.dram_tensor("yT", [H, S_E], BF16)
    myslice = nc.dram_tensor("myslice", [S_NODE, P], BF16)
    sendbuf = nc.dram_tensor("sendbuf", [NCORE * R, P], BF16)
    recvbuf = nc.dram_tensor("recvbuf", [NCORE * R, P], BF16)

    myslice_pview = myslice.ap().rearrange("(b p) f -> p b f", p=P)
    outslots_pview = outslots.ap().rearrange("(b p) o -> p b o", p=P)

    ACT = mybir.ActivationFunctionType
    ALU = mybir.AluOpType

    with tile.TileContext(nc) as tc:
        with (
            tc.tile_pool(name="persist", bufs=1) as pp,
            tc.tile_pool(name="work", bufs=3) as wp,
            tc.tile_pool(name="hpool", bufs=3) as hq,
            tc.tile_pool(name="sendp", bufs=2) as sp,
            tc.tile_pool(name="callbuf", bufs=16) as cbp,
            tc.tile_pool(name="pza", bufs=4, space="PSUM") as pza,
            tc.tile_pool(name="pzb", bufs=2, space="PSUM") as pzb,
            tc.tile_pool(name="ptrp", bufs=2, space="PSUM") as ptrp,
        ):
            # ---- persistent tiles ----
            identb = pp.tile([P, P], BF16, tag="identb")
            make_identity(nc, identb[:])
            W = {}
            for name, shp in wshapes.items():
                t = pp.tile(list(shp), _WDTYPES[name], tag=f"w_{name}")
                nc.sync.dma_start(out=t[:], in_=din[name][:, :])
                W[name] = t
            xt = pp.tile([P, S_NODE], F32, tag="xt")      # rows 64:128 = x~
            xbf = pp.tile([P, S_NODE], BF16, tag="xbf")   # rows 64:128 = bf16(x~)
            staging = pp.tile([P, NBLK * H], BF16, tag="staging")
            staging2 = pp.tile([P, NBLK], F32, tag="staging2")
            sidx = pp.tile([P, S_E // 16], I16, tag="sidx")
            nc.sync.dma_start(out=sidx[:], in_=slotidx[:, :])
            kidx = pp.tile([P, NCORE * R // 16], I16, tag="kidx")
            nc.sync.dma_start(out=kidx[:], in_=sendidx[:, :])
            msk = pp.tile([P, NMASK * P], F32, tag="msk")
            nc.sync.dma_start(out=msk[H:P, :], in_=mask64[:, :])

            # zero myslice's pad half once (gathered but never consumed)
            nc.vector.memset(staging[:, :], 0.0)
            nc.sync.dma_start(
                out=myslice_pview[:, :, H:P],
                in_=staging[:].rearrange("p (b f) -> p b f", b=NBLK))

            evac_ct = [0]

            def evac(dst_ap, src_ap):
                if evac_ct[0] % 2 == 0:
                    nc.scalar.copy(out=dst_ap, in_=src_ap)
                else:
                    nc.vector.tensor_copy(out=dst_ap, in_=src_ap)
                evac_ct[0] += 1

            # ---------- x~ scatter-max consume (with mask on flagged subtiles) ----------
            def consume(col0, w, sbase, w2p):
                j = 0
                while j < w // P:
                    gsub = (sbase + j * P) // P
                    if gsub in mask_index:
                        mi = mask_index[gsub]
                        tmp = wp.tile([P, P], F32, tag="mtmp")
                        nc.vector.tensor_tensor(
                            out=tmp[H:P, :], in0=w2p[H:P, j * P:(j + 1) * P],
                            in1=msk[H:P, mi * P:(mi + 1) * P], op=ALU.add)
                        nc.vector.tensor_tensor(
                            out=xt[H:P, col0 + j * P:col0 + (j + 1) * P],
                            in0=xt[H:P, col0 + j * P:col0 + (j + 1) * P],
                            in1=tmp[H:P, :], op=ALU.max)
                        j += 1
                    else:
                        j2 = j
                        while j2 < w // P and ((sbase + j2 * P) // P) not in mask_index:
                            j2 += 1
                        nc.vector.tensor_tensor(
                            out=xt[H:P, col0 + j * P:col0 + j2 * P],
                            in0=xt[H:P, col0 + j * P:col0 + j2 * P],
                            in1=w2p[H:P, j * P:j2 * P], op=ALU.max)
                        j = j2

            # ---------- readback: x~ -> xbf -> myslice (transposed bf16) ----------
            def readback():
                nc.vector.tensor_copy(out=xbf[H:P, :], in_=xt[H:P, :])
                for b in range(NBLK):
                    ps = ptrp.tile([P, TILE_W], F32, tag="ptr")
                    psb = ps[:].bitcast(BF16)[:, 0:H]
                    nc.tensor.transpose(
                        out=psb, in_=xbf[H:P, b * P:(b + 1) * P],
                        identity=identb[H:P, H:P])
                    evac(staging[:, b * H:(b + 1) * H], psb)
                nc.sync.dma_start(
                    out=myslice_pview[:, :, 0:H],
                    in_=staging[:].rearrange("p (b f) -> p b f", b=NBLK))

            # queue_num must track the global Pool-DMA instruction order:
            # tile_sem_assignment rotates DMASW sem lanes per instruction and
            # each lane is serviced by the matching SWDGE queue.
            gq = [0]

            def next_q():
                q = gq[0] % 4
                gq[0] += 1
                return q

            # ---------- exchange: myslice -> sendbuf -> A2A -> recvbuf ----------
            def exchange():
                for dd in range(NCORE):
                    st = sp.tile([P, RJ, P], BF16, tag="sendt")
                    off = 0
                    while off < R:
                        n = min(GC, R - off)
                        nc.gpsimd.dma_gather(
                            out_ap=st[:, off // P:(off + n) // P, :],
                            in_ap=myslice[:, :],
                            idxs_ap=kidx[:, (dd * R + off) // 16:(dd * R + off + n) // 16],
                            num_idxs=n, num_idxs_reg=n, elem_size=P,
                            queue_num=next_q())
                        off += n
                    dv = sendbuf.ap()[dd * R:(dd + 1) * R, :].rearrange(
                        "(p j) f -> p (j f)", p=P)
                    nc.sync.dma_start(out=dv, in_=st[:].rearrange("p j f -> p (j f)"))
                if SIM_SINGLE:
                    nc.sync.dma_start(out=recvbuf.ap().rearrange(
                        "(p a) f -> p (a f)", p=P),
                        in_=sendbuf.ap().rearrange("(p a) f -> p (a f)", p=P))
                else:
                    nc.gpsimd.collective_compute(
                        "AllToAll", ALU.bypass,
                        replica_groups=[list(range(NCORE))],
                        ins=[sendbuf.ap().bitcast(F32)],
                        outs=[recvbuf.ap().bitcast(F32)])

            # ---------- slot gather: recvbuf -> callbuf chunks (row-major) ----------
            def slot_gather():
                bufs = []
                base = 0
                while base < S_E:
                    n = min(GC, S_E - base)
                    st = cbp.tile([P, GC // P, P], BF16, tag="cb")
                    nc.gpsimd.dma_gather(
                        out_ap=st[:, : n // P, :], in_ap=recvbuf[:, :],
                        idxs_ap=sidx[:, base // 16:(base + n) // 16],
                        num_idxs=n, num_idxs_reg=n, elem_size=P,
                        queue_num=next_q())
                    bufs.append(st)
                    base += n
                return bufs

            # ---------- per-tile stacked rhs: rows 0:64 = xj~^T (PE-transposed
            # gathered blocks), rows 64:128 = xi~ (aligned copy of xbf) ----------
            def make_stk(bufs, col0, sbase, w):
                stk = wp.tile([P, TILE_W], BF16, tag="stk")
                for j in range(w // P):
                    s = sbase + j * P
                    g = bufs[s // GC][:, (s % GC) // P, 0:H]
                    ps = ptrp.tile([P, TILE_W], F32, tag="ptr")
                    psb = ps[:].bitcast(BF16)[0:H, 0:P]
                    nc.tensor.transpose(out=psb, in_=g, identity=identb[:, :])
                    evac(stk[0:H, j * P:(j + 1) * P], psb)
                nc.sync.dma_start(out=stk[H:P, :w],
                                  in_=xbf[H:P, col0:col0 + w])
                return stk

            # ---------- fused phase: fy(k) then fx(k), sharing gathered x ----------
            def fused_phase(k, bufs):
                KSUB = int(os.environ.get("KSUB", "9"))
                with_fy = k > 0
                write_y = k < LOOP - 1
                hp_cur = None     # [h1y(t); h1x(t-1)]
                pend = None       # (col0, w, sbase) of tile t-1 awaiting z2x
                for ti, (r, col0, w, sbase) in enumerate(tiles):
                    if ti >= int(os.environ.get("KTILES", "9999")):
                        break
                    stk = make_stk(bufs, col0, sbase, w)
                    if KSUB < 1:
                        continue
                    z = pza.tile([P, TILE_W], F32, tag="z")
                    nc.tensor.matmul(z[:, :w], W["wAB"][:], stk[:, :w],
                                     start=True, stop=True)
                    if KSUB < 2:
                        continue
                    yt = wp.tile([H, TILE_W], BF16, tag="yt")
                    nc.sync.dma_start(out=yt[:, :w], in_=yT[:, sbase:sbase + w])
                    if with_fy:
                        if hp_cur is None:
                            hp_cur = hq.tile([P, TILE_W], BF16, tag="hp")
                            nc.vector.memset(hp_cur[H:P, :], 0.0)
                        nc.scalar.activation(out=hp_cur[0:H, :w], in_=z[0:H, :w],
                                             func=ACT.Relu,
                                             bias=W["b1pair"][0:H, 0:1])
                        wz = max(w, pend[1]) if pend is not None else w
                        if pend is not None and pend[1] > w:
                            # h1y(t) gap: z2 streams wz cols, relu wrote only w
                            nc.vector.memset(hp_cur[0:H, w:pend[1]], 0.0)
                        if pend is not None and w > pend[1]:
                            # h1x(t-1) gap: written only to pend[1]
                            nc.vector.memset(hp_cur[H:P, pend[1]:w], 0.0)
                        w2p = pzb.tile([P, TILE_W], F32, tag="w2p")
                        nc.tensor.matmul(w2p[:, :wz], W["w2diag"][:],
                                         hp_cur[:, :wz], start=True, stop=True)
                        # y~ = max(y~, z2y(t))
                        nc.vector.tensor_tensor(out=yt[:, :w], in0=yt[:, :w],
                                                in1=w2p[0:H, :w], op=ALU.max)
                        if write_y:
                            nc.sync.dma_start(out=yT[:, sbase:sbase + w],
                                              in_=yt[:, :w])
                        if pend is not None:
                            consume(pend[0], pend[1], pend[2], w2p)
                        pend = (col0, w, sbase)
                    nc.tensor.matmul(z[H:P, :w], W["fxC"][:], yt[:, :w],
                                     start=False, stop=True, skip_group_check=True)
                    if KSUB < 3:
                        continue
                    hp_next = hq.tile([P, TILE_W], BF16, tag="hp")
                    nc.scalar.activation(out=hp_next[H:P, :w], in_=z[H:P, :w],
                                         func=ACT.Relu, bias=W["b1pair"][H:P, 0:1])
                    if KSUB < 4:
                        continue
                    if not with_fy:
                        w2p = pzb.tile([P, TILE_W], F32, tag="w2p")
                        nc.tensor.matmul(w2p[H:P, :w], W["fxw2_hi"][H:P, :],
                                         hp_next[H:P, :w], start=True, stop=True)
                        consume(col0, w, sbase, w2p)
                    hp_cur = hp_next
                if with_fy:
                    # flush: z2x of the last tile
                    lc, lw, lsb = pend
                    w2p = pzb.tile([P, TILE_W], F32, tag="w2p")
                    nc.tensor.matmul(w2p[H:P, :lw], W["fxw2_hi"][H:P, :],
                                     hp_cur[H:P, :lw], start=True, stop=True)
                    consume(lc, lw, lsb, w2p)

            # ---------- init: hx (f32) ----------
            K36 = wshapes["hx_w1"][0]
            off = 0
            while off < S_NODE:
                w = min(TILE_W, S_NODE - off)
                ft = wp.tile([K36, TILE_W], F32, tag="ft")
                nc.sync.dma_start(out=ft[:, :w], in_=feat36T[:, off:off + w])
                z1 = pza.tile([P, TILE_W], F32, tag="z")
                nc.tensor.matmul(z1[0:H, :w], W["hx_w1"][:], ft[:, :w],
                                 start=True, stop=True)
                h1 = wp.tile([H, TILE_W], F32, tag="h1")
                nc.scalar.activation(out=h1[:, :w], in_=z1[0:H, :w],
                                     func=ACT.Relu, bias=W["hx_b1"][:, 0:1])
                z2 = pza.tile([P, TILE_W], F32, tag="z")
                nc.tensor.matmul(z2[H:P, :w], W["hx_w2"][:], h1[:, :w],
                                 start=True, stop=True)
                nc.scalar.activation(out=xt[H:P, off:off + w], in_=z2[H:P, :w],
                                     func=ACT.Identity, bias=W["hxb2"][H:P, 0:1])
                off += w

            # ---------- initial exchange of x0 (issued before hy init so the
            # send-gathers + AllToAll overlap hy's PE/ACT work) ----------
            KSTAGE = int(os.environ.get("KSTAGE", "0"))
            readback()
            exchange()

            # ---------- init: hy (bf16, one pipelined pass per tile) ----------
            # pass t: z1(t) (rows 64:128) from rt[0:18]=r18(t);
            #         z2(t-1) (rows 0:64) from rt[64:128]=h1y(t-1)
            rt_cur = wp.tile([P, TILE_W], BF16, tag="rt")
            nc.vector.memset(rt_cur[:, :], 0.0)
            nc.sync.dma_start(out=rt_cur[0:K18, :tiles[0][2]],
                              in_=rhs18[:, 0:tiles[0][2]])
            pw = 0
            psb_prev = 0
            for ti, (r, col0, w, sbase) in enumerate(tiles):
                wz = max(w, pw)
                zi = pza.tile([P, TILE_W], F32, tag="z")
                nc.tensor.matmul(zi[:, :wz], W["hyw_comb"][:], rt_cur[:, :wz],
                                 start=True, stop=True)
                if ti > 0:
                    yt0 = wp.tile([H, TILE_W], BF16, tag="yt")
                    nc.scalar.activation(out=yt0[:, :pw], in_=zi[0:H, :pw],
                                         func=ACT.Identity, bias=W["hyb"][0:H, 0:1])
                    nc.sync.dma_start(out=yT[:, psb_prev:psb_prev + pw],
                                      in_=yt0[:, :pw])
                if ti + 1 < len(tiles):
                    nw = tiles[ti + 1][2]
                    rt_next = wp.tile([P, TILE_W], BF16, tag="rt")
                    nc.vector.memset(rt_next[:, :], 0.0)
                    nc.sync.dma_start(
                        out=rt_next[0:K18, :nw],
                        in_=rhs18[:, tiles[ti + 1][3]:tiles[ti + 1][3] + nw])
                    nc.scalar.activation(out=rt_next[H:P, :w], in_=zi[H:P, :w],
                                         func=ACT.Relu, bias=W["hyb"][H:P, 0:1])
                    rt_cur = rt_next
                else:
                    # flush: z2 of the last tile via hy_w2-only pass
                    hlast = wp.tile([P, TILE_W], BF16, tag="rt")
                    nc.scalar.activation(out=hlast[H:P, :w], in_=zi[H:P, :w],
                                         func=ACT.Relu, bias=W["hyb"][H:P, 0:1])
                    zf = pza.tile([P, TILE_W], F32, tag="z")
                    nc.tensor.matmul(zf[0:H, :w], W["hyw2_hi"][H:P, :],
                                     hlast[H:P, :w], start=True, stop=True)
                    ytf = wp.tile([H, TILE_W], BF16, tag="yt")
                    nc.scalar.activation(out=ytf[:, :w], in_=zf[0:H, :w],
                                         func=ACT.Identity, bias=W["hyb"][0:H, 0:1])
                    nc.sync.dma_start(out=yT[:, sbase:sbase + w], in_=ytf[:, :w])
                pw = w
                psb_prev = sbase

            # ---------- iterations ----------
            if KSTAGE != 1:
                for k in range(LOOP):
                    bufs = slot_gather()
                    if KSTAGE == 2:
                        break
                    fused_phase(k, bufs)
                    if KSTAGE == 3:
                        break
                    if k < LOOP - 1:
                        readback()
                        exchange()

            # ---------- final MLP (f32) ----------
            off = 0
            while off < S_NODE:
                w = min(TILE_W, S_NODE - off)
                z1 = pza.tile([P, TILE_W], F32, tag="z")
                nc.tensor.matmul(z1[0:H, :w], W["feta_w1_hi"][H:P, :],
                                 xt[H:P, off:off + w], start=True, stop=True)
                h1 = wp.tile([H, TILE_W], F32, tag="h1")
                nc.scalar.activation(out=h1[:, :w], in_=z1[0:H, :w],
                                     func=ACT.Relu, bias=W["feta_b1e"][:, 0:1])
                z2 = pza.tile([P, TILE_W], F32, tag="z")
                nc.tensor.matmul(z2[0:H, :w], W["feta_w2"][:], h1[:, :w],
                                 start=True, stop=True)
                h2 = wp.tile([H, TILE_W], F32, tag="h2")
                nc.scalar.activation(out=h2[:, :w], in_=z2[0:H, :w],
                                     func=ACT.Relu, bias=W["feta_b2"][:, 0:1])
                for j in range(w // P):
                    b = (off + j * P) // P
                    ps = pzb.tile([P, TILE_W], F32, tag="w2p")
                    nc.tensor.matmul(ps[:, 0:1], h2[:, j * P:(j + 1) * P],
                                     W["feta_w3"][:], start=True, stop=True)
                    evac(staging2[:, b:b + 1], ps[:, 0:1])
                off += w
            nc.sync.dma_start(
                out=outslots_pview,
                in_=staging2[:].rearrange("p (b o) -> p b o", b=NBLK))

    _log(f"built program: {S_E=} {len(tiles)=} masks={NMASK} R={R}")
    nc.compile()
    _log("compiled")
    return nc


def kernel(**inputs):
    global LAST_EXEC_NS, LAST_TRACE
    v = np.asarray(inputs["v"], dtype=np.float32)
    labels = np.asarray(inputs["labels"], dtype=np.float32)
    edge_index = np.asarray(inputs["edge_index"]).astype(np.int64)
    LOOP = int(np.asarray(inputs["loop"]))

    import hashlib
    ck = hashlib.sha1(edge_index.tobytes()).hexdigest() + f"_{LOOP}_{v.shape}"
    if ck in _BUILD_CACHE:
        meta, pci, nc = _BUILD_CACHE[ck]
    else:
        meta, pci, nc = None, None, None
    if meta is None:
        meta, pci = _preprocess(v, labels, edge_index)
    wf = _fold_weights({k: np.asarray(val, dtype=np.float32)
                        for k, val in inputs.items()
                        if k not in ("v", "labels", "edge_index", "loop")})
    wnames = list(_WDTYPES.keys())
    wshapes = {n: wf[n].shape for n in wnames}

    if nc is None:
        nc = _build(meta, wshapes, LOOP)
        _BUILD_CACHE[ck] = (meta, pci, nc)

    in_maps = []
    for c in range(NCORE):
        m = {n: wf[n] for n in wnames}
        m["feat36T"] = pci[c]["feat36T"]
        m["rhs18"] = pci[c]["rhs18"]
        m["slotidx"] = pci[c]["slotidx"]
        m["sendidx"] = pci[c]["sendidx"]
        m["mask64"] = pci[c]["mask64"]
        in_maps.append(m)

    res = run_bass_kernel_spmd(nc, in_maps, core_ids=list(range(NCORE)),
                               tmpdir=os.environ.get("BASS_TMPDIR") or None)
    LAST_EXEC_NS = res.exec_time_ns
    LAST_TRACE = res.instructions_and_trace

    N = meta["N"]
    out = np.zeros((N, 1), dtype=np.float32)
    for c in range(NCORE):
        cc = meta["cores"][c]
        slots = cc["slot_of_local"]
        vals = res.results[c]["outslots"][:, 0]
        out[cc["lo"]:cc["lo"] + cc["nloc"], 0] = vals[slots]
    return out
